# revision 94
# baseline (speedup 1.0000x reference)
"""Multi-head attention (B=4, S=2048, E=768, H=8, D=96) on 8 Trainium2 cores.

Sharding: core c -> (batch b = c//2, head-group hg = c%2 of 4 heads).
Each core computes Q/K/V projections for its 4 heads over the full sequence
of its batch, full attention for those heads, and a partial output
projection (row-split Wo).  The two cores of a batch produce partial
outputs that are summed on the host during unsharding (tensor-parallel
reduce).

On-chip layout notes:
  - All matmul operands are bf16 (1 cycle/row on PE; fp32 would be 4x; fp8
    DoubleRow would halve PE time but its ~3% per-element noise lands ~1:1
    in the output and busts the 2e-2 tolerance).
  - Scores are computed transposed, S^T[k, q] = K^T.T @ Q^T (the K/Q
    projections pad head dim 96->128 on the contraction partitions, which
    costs nothing).  exp(S) runs on ACT straight out of PSUM over a
    [128, 1024] pair of key tiles with the 1/sqrt(d) scale folded in.
    ACT's 1038 ns per pair is the cadence ceiling for bare windows.
  - The O = V^T exp(S) matmul is FLIPPED: the exp tile is the stationary
    operand (Ldweights is free) and a 97-column vaug slab (96 V dims + a
    ones column) streams, so each matmul costs 40 ns instead of 213 and
    the per-pair PE work drops from 852 to 749 ns.  The ones column makes
    the softmax denominator fall out at column 96 of each q-tile group of
    the [q, 4 x 97] accumulator, so normalization is a per-partition
    reciprocal + tensor_scalar - no broadcast machinery at all.
  - A PSUM bank supports only ONE open accumulation group at a time, so
    the flipped accumulator (4 interleaved chains in one bank) is zeroed
    up front and every matmul accumulates with start=False.
  - The normalized [q, d] output is transposed back to the [d, q] layout
    the output projection needs via 128x128 PE transposes (53 ns each)
    against a host-provided identity.  attnT chunks t0/t1 only contain
    heads 0-2, so they repack during head 3's own windows (the ACT-bound
    first window absorbs three q-chunks' worth as PE filler); only the t2
    chunk waits for head 3's normalization, which keeps the tail short.
  - O matmuls consume exps with a 4-pair lag so the ACT exp latency
    (~1.35 us from S matmul to usable exp) never stalls the PE.  Both S
    matmuls of a pair are emitted BEFORE the exp that reads them: Tile's
    conservative subtile tracking would otherwise serialize the second
    matmul behind the exp (false intra-tile WAR).
  - Inputs are host-packed so each operand is ONE [128, n] DMA.  x is
    packed seq-block-major; the first x half rides the Pool/SWDGE queue
    (independent of the HWDGE slot) so it overlaps the wk0 load; head 0's
    K/Q weight columns load before the other heads'.
  - PE stream order: head 0's K/Q projections and V chunks per x block,
    then 16 attention windows (4 heads x 4 q-chunks).  Heads 0-2 carry
    the next head's K/Q chunks as thunked PE filler; head 3's windows
    carry the previous q-chunk's t2 repack + two-phase output projection
    (P = t0/t1 partials, F = t2 close + DVE evictions + stores split
    across the SP/ACT/SWDGE queues).  PSUM rings: pss pairs (2x2 banks),
    flipped-O accumulators (2x1), projection/repack ring (2x1); every
    allocation is placed one eviction behind its ring slot's previous
    user.
"""

import os
import sys

sys.path.insert(0, "/opt/trn_rl_repo")

import numpy as np
import ml_dtypes

import concourse.bacc as bacc
import concourse.tile as tile
from concourse import mybir
from concourse.bass_utils import run_bass_kernel_spmd

BF16 = ml_dtypes.bfloat16

EMB = 768
HEADS = 8
HD = 96          # true head dim
HDP = 128        # padded head dim
SEQ = 2048
B = 4
NCORES = 8
HPC = 4          # heads per core
SCALING = HD ** -0.5
QC = 512         # query chunk per attention inner loop
NQC = SEQ // QC
NKT = SEQ // 128  # 16 key tiles
NPAIR = NKT // 2
NE = EMB // 128   # 6 e_in tiles

_NC_CACHE = {}
LAST_RESULT = None  # BassKernelResults of the most recent run (for test.py)


def _build_nc():
    f32 = mybir.dt.float32
    bf = mybir.dt.bfloat16

    nc = bacc.Bacc(trn_type="TRN2", target_bir_lowering=False, debug=False,
                   num_devices=NCORES)

    # All operands host-packed into [128, n] so each loads as ONE DMA.
    xtp = nc.dram_tensor("xtp", [128, NE * SEQ], bf, kind="ExternalInput").ap()
    # K/Q weights split: head 0's columns load first (0.19MB) so head 0's
    # projections - which gate the whole pipeline - start ~5 us earlier
    wqp0 = nc.dram_tensor("wqp0", [128, NE * HDP], bf,
                          kind="ExternalInput").ap()
    wqpr = nc.dram_tensor("wqpr", [128, NE * 3 * HDP], bf,
                          kind="ExternalInput").ap()
    wkp0 = nc.dram_tensor("wkp0", [128, NE * HDP], bf,
                          kind="ExternalInput").ap()
    wkpr = nc.dram_tensor("wkpr", [128, NE * 3 * HDP], bf,
                          kind="ExternalInput").ap()
    wvp = nc.dram_tensor("wvp", [128, NE * HPC * HD], bf,
                         kind="ExternalInput").ap()
    wop = nc.dram_tensor("wop", [128, 3 * EMB], bf, kind="ExternalInput").ap()
    bqp = nc.dram_tensor("bqp", [128, HPC], f32, kind="ExternalInput").ap()
    bkp = nc.dram_tensor("bkp", [128, HPC], f32, kind="ExternalInput").ap()
    identp = nc.dram_tensor("identp", [128, 128], bf,
                            kind="ExternalInput").ap()
    outp = nc.dram_tensor("outp", [SEQ, EMB], f32, kind="ExternalOutput").ap()

    with tile.TileContext(nc) as tc:
        with (
            tc.tile_pool(name="const", bufs=1) as constp,
            tc.tile_pool(name="big", bufs=1) as bigp,
            tc.tile_pool(name="expp", bufs=6) as expp,
            tc.tile_pool(name="rbp", bufs=4) as rbp,
            tc.tile_pool(name="normp", bufs=1) as normp,
            tc.tile_pool(name="outsb", bufs=12) as outsb,
            tc.tile_pool(name="ps_proj", bufs=2, space="PSUM") as ps_proj,
            tc.tile_pool(name="ps_o", bufs=2, space="PSUM") as ps_o,
            tc.tile_pool(name="ps_pair", bufs=2, space="PSUM") as ps_pair,
        ):
            # ---- loads. x is packed seq-block-major ([128, 6e x 512] per
            # 512-sequence block) so the first K/Q chunk only needs wk + one
            # 0.75MB block; wk + block 0/1 load first, k/q chunks of head 0
            # then pipeline behind the remaining block DMAs. ----
            XB = NE * 512  # 3072 cols per seq block
            wk0_sb = constp.tile([128, NE * HDP], bf, name="wk0_sb")
            nc.sync.dma_start(out=wk0_sb, in_=wkp0)
            xtb = [bigp.tile([128, XB], bf, name=f"xtb{n}") for n in range(4)]
            # the first x half rides the Pool/SWDGE queue: its descriptor
            # path is independent of the HWDGE slot the weight loads use,
            # so it overlaps wk0 end-to-end
            nc.gpsimd.dma_start(out=xtb[0][:, 0:XB // 2],
                                in_=xtp[:, 0:XB // 2])
            nc.sync.dma_start(out=xtb[0][:, XB // 2:XB],
                              in_=xtp[:, XB // 2:XB])
            # biases next: tiny, but they gate the K/Q psum evictions
            bq_sb = constp.tile([128, HPC], f32, name="bq_sb")
            nc.sync.dma_start(out=bq_sb, in_=bqp)
            bk_sb = constp.tile([128, HPC], f32, name="bk_sb")
            nc.sync.dma_start(out=bk_sb, in_=bkp)
            wq0_sb = constp.tile([128, NE * HDP], bf, name="wq0_sb")
            nc.sync.dma_start(out=wq0_sb, in_=wqp0)
            wv_sb = constp.tile([128, NE * HPC * HD], bf, name="wv_sb")
            nc.sync.dma_start(out=wv_sb, in_=wvp)
            nc.sync.dma_start(out=xtb[1], in_=xtp[:, XB:2 * XB])
            wkr_sb = constp.tile([128, NE * 3 * HDP], bf, name="wkr_sb")
            nc.sync.dma_start(out=wkr_sb, in_=wkpr)
            wqr_sb = constp.tile([128, NE * 3 * HDP], bf, name="wqr_sb")
            nc.sync.dma_start(out=wqr_sb, in_=wqpr)
            nc.sync.dma_start(out=xtb[2], in_=xtp[:, 2 * XB:3 * XB])
            nc.sync.dma_start(out=xtb[3], in_=xtp[:, 3 * XB:4 * XB])
            wo_sb = constp.tile([128, 3 * EMB], bf, name="wo_sb")
            nc.sync.dma_start(out=wo_sb, in_=wop)
            # identity for the PE repack transposes (first needed in head
            # 3's phase, so it loads last)
            ident_sb = constp.tile([128, 128], bf, name="ident_sb")
            nc.sync.dma_start(out=ident_sb, in_=identp)

            def wk_eh(e, h):
                if h == 0:
                    return wk0_sb[:, e * HDP:(e + 1) * HDP]
                return wkr_sb[:, (e * 3 + h - 1) * HDP:(e * 3 + h) * HDP]

            def wq_eh(e, h):
                if h == 0:
                    return wq0_sb[:, e * HDP:(e + 1) * HDP]
                return wqr_sb[:, (e * 3 + h - 1) * HDP:(e * 3 + h) * HDP]

            def wv_e(e):
                return wv_sb[:, e * HPC * HD:(e + 1) * HPC * HD]

            def wo_t(t_):
                return wo_sb[:, t_ * EMB:(t_ + 1) * EMB]

            # ---- persistent intermediates ----
            # vaug: per key tile, 4 heads x (96 v-cols + a ones col).  The
            # ones col makes row^T @ vaug yield the softmax denominator in
            # the same accumulator (col 96 of each head group).  No pad
            # cols: vaug is the MOVING operand of the flipped O matmul, so
            # narrower means cheaper (97 vs 128 cycles).
            HDV = HD + 1
            vaug = []
            for kt in range(NKT):
                t = bigp.tile([128, HPC * HDV], bf, name=f"vaug{kt}")
                ones_cols = t.rearrange("p (h c) -> p h c",
                                        h=HPC)[:, :, HD:HD + 1]
                nc.gpsimd.memset(ones_cols, 1.0)
                vaug.append(t)
            qT = [bigp.tile([128, SEQ], bf, name=f"qT{h}") for h in range(HPC)]
            kT = [bigp.tile([128, SEQ], bf, name=f"kT{h}") for h in range(HPC)]
            # packed attention output, [384 rows = 3 tiles x 128, seq],
            # written by the h3-phase repack transposes
            attnT = [bigp.tile([128, SEQ], bf, name=f"attnT{t_}")
                     for t_ in range(3)]
            # normalized O in [q, head-dims] layout, per (q-chunk, q-tile):
            # written per head as its window completes, transposed into
            # attnT once all four heads are in
            normed = [[normp.tile([128, HPC * HD], bf, name=f"nm{qc}_{qt}")
                       for qt in range(4)] for qc in range(NQC)]

            f32_ = f32

            # ---- projection emit helpers ----
            def emit_v_chunk(kt):
                psv = ps_proj.tile([128, 512], f32_, tag="ps",
                                   name=f"psv{kt}")
                blk, off = divmod(kt, 4)
                for e in range(NE):
                    nc.tensor.matmul(psv[:, 0:HPC * HD],
                                     lhsT=xtb[blk][:, e * 512 + off * 128:
                                                   e * 512 + off * 128 + 128],
                                     rhs=wv_e(e),
                                     start=(e == 0), stop=(e == NE - 1))
                for hh in range(HPC):
                    nc.vector.tensor_copy(
                        vaug[kt][:, hh * HDV:hh * HDV + HD],
                        psv[:, hh * HD:(hh + 1) * HD])

            kq_ps = {}

            def emit_kq_part(h, n, which, part):
                """One half of a K/Q chunk (3 of 6 e-matmuls).  Thunking
                chunks at this granularity keeps the per-pair PE load above
                the ACT exp cadence, so the S stream never outruns ACT into
                the 2-slot pss ring."""
                key = (h, n, which)
                w_eh, dst, b_sb = ((wk_eh, kT, bk_sb) if which == "k"
                                   else (wq_eh, qT, bq_sb))
                if part == 0:
                    kq_ps[key] = ps_proj.tile([128, 512], f32_, tag="ps",
                                              name=f"ps{which}{h}_{n}")
                ps = kq_ps[key]
                for e in (3 * part, 3 * part + 1, 3 * part + 2):
                    nc.tensor.matmul(ps,
                                     lhsT=w_eh(e, h),
                                     rhs=xtb[n][:, e * 512:(e + 1) * 512],
                                     start=(e == 0), stop=(e == NE - 1))
                if part == 1:
                    nsl = slice(n * 512, (n + 1) * 512)
                    nc.vector.tensor_scalar_add(dst[h][:, nsl],
                                                kq_ps.pop(key),
                                                b_sb[:, h:h + 1])

            def emit_kq_chunk(h, n, which):
                for part in range(2):
                    emit_kq_part(h, n, which, part)

            def kq_chunks(h):
                for n in range(4):
                    yield ("k", h, n)
                    yield ("q", h, n)

            # ---- output projection chunk (one 128-row q tile) ----
            # Split across two 1-bank psums so it can borrow ps_proj slots;
            # PSUM->SBUF copies go on DVE (ACT is busy with exp here).
            # Tail out-proj chunks, two-phase: head 3's rows live only in
            # packed attnT tile 2, so the t0/t1 matmuls are independent of
            # the final normalization and run DURING its chain; only the t2
            # matmul (accumulation close) waits for the per-qm norm slice.
            # Psums borrow the attention rings (free by then).
            tail_ps = {}

            def emit_out_partial(qm, mode="tail"):
                qsl = slice(qm * 128, (qm + 1) * 128)
                if mode == "win":
                    psA = ps_proj.tile([128, 512], f32_, tag="ps",
                                       name=f"poA{qm}")
                    psB = ps_proj.tile([128, 512], f32_, tag="ps",
                                       name=f"poB{qm}")
                elif mode == "edge":
                    # last-window partial: psA on the ps ring, psB on the
                    # pso ring's spare slot (the previous psof freed once
                    # its normalization drained) so only ONE ps slot is
                    # held across the window/tail boundary
                    psA = ps_proj.tile([128, 512], f32_, tag="ps",
                                       name=f"poA{qm}")
                    psB = ps_o.tile([128, 512], f32_, tag="pso",
                                    name=f"poB{qm}")
                else:
                    # tail: both halves on the pss ring (free then)
                    psA = ps_pair.tile([128, 512], f32_, tag="pss",
                                       name=f"poA{qm}")
                    psB = ps_pair.tile([128, 512], f32_, tag="pss",
                                       name=f"poB{qm}")
                tail_ps[qm] = (psA, psB)
                for t in range(2):
                    nc.tensor.matmul(psA,
                                     lhsT=attnT[t][:, qsl],
                                     rhs=wo_t(t)[:, 0:512],
                                     start=(t == 0), stop=False)
                    nc.tensor.matmul(psB[:, 0:256],
                                     lhsT=attnT[t][:, qsl],
                                     rhs=wo_t(t)[:, 512:768],
                                     start=(t == 0), stop=False)

            def emit_out_finish(qm, windowed=False):
                qsl = slice(qm * 128, (qm + 1) * 128)
                psA, psB = tail_ps.pop(qm)
                nc.tensor.matmul(psA, lhsT=attnT[2][:, qsl],
                                 rhs=wo_t(2)[:, 0:512],
                                 start=False, stop=True)
                nc.tensor.matmul(psB[:, 0:256], lhsT=attnT[2][:, qsl],
                                 rhs=wo_t(2)[:, 512:768],
                                 start=False, stop=True)
                out_sb = outsb.tile([128, EMB], f32_, tag="osb",
                                    name=f"osb{qm}")
                # evictions on DVE in windows (ACT is exp-saturated there;
                # GPSIMD cannot touch PSUM on real hw).  In the tail, psB
                # rides ACT end-to-end (evict + store) so the last stores
                # split across the SP and ACT queues instead of
                # serializing on SP's 650 ns issue cost.
                nc.vector.tensor_copy(out_sb[:, 0:512], psA)
                if windowed:
                    nc.sync.dma_start(
                        out=outp[qm * 128:(qm + 1) * 128, 0:512],
                        in_=out_sb[:, 0:512])
                else:
                    eng = nc.gpsimd if qm % 2 == 0 else nc.sync
                    eng.dma_start(
                        out=outp[qm * 128:(qm + 1) * 128, 0:512],
                        in_=out_sb[:, 0:512])
                if windowed:
                    nc.vector.tensor_copy(out_sb[:, 512:768], psB[:, 0:256])
                    nc.sync.dma_start(
                        out=outp[qm * 128:(qm + 1) * 128, 512:768],
                        in_=out_sb[:, 512:768])
                else:
                    # tail: psB evict on DVE; store rotated across the
                    # ACT and Pool/SWDGE queues so the four final stores
                    # don't serialize on the single HWDGE slot
                    nc.vector.tensor_copy(out_sb[:, 512:768], psB[:, 0:256])
                    eng = nc.scalar if qm % 2 == 0 else nc.gpsimd
                    eng.dma_start(
                        out=outp[qm * 128:(qm + 1) * 128, 512:768],
                        in_=out_sb[:, 512:768])

            # ---- attention emit (with interleaved PE filler work) ----
            # repack: transpose normed[qc][qt] ([128 q, 384 packed head
            # dims]) into attnT via 128x128 PE transposes + DVE evictions.
            # Chunks t0/t1 cover only heads 0-2, so they repack as soon as
            # head 2's normalization lands - during head 3's own windows -
            # and the out-proj partials (which read t0/t1) can then run
            # inside the last window.  Only the t2 chunk (heads 2+3) waits
            # for head 3's normalization.
            def emit_repack01(qc, qt):
                tps = ps_proj.tile([128, 2 * 128], bf, tag="ps",
                                   name=f"tp01_{qc}_{qt}")
                for t_ in range(2):
                    nc.tensor.transpose(
                        tps[:, t_ * 128:(t_ + 1) * 128],
                        normed[qc][qt][:, t_ * 128:(t_ + 1) * 128],
                        ident_sb)
                csl = slice(qc * QC + qt * 128, qc * QC + (qt + 1) * 128)
                for t_ in range(2):
                    nc.vector.tensor_copy(attnT[t_][:, csl],
                                          tps[:, t_ * 128:(t_ + 1) * 128])

            def emit_repack2(qc, qt, tail=False):
                tps = ps_proj.tile([128, 128], bf, tag="ps",
                                   name=f"tp2_{qc}_{qt}")
                nc.tensor.transpose(tps, normed[qc][qt][:, 256:384],
                                    ident_sb)
                csl = slice(qc * QC + qt * 128, qc * QC + (qt + 1) * 128)
                if tail:
                    # keep the tail's DVE queue clear for the out evictions
                    nc.scalar.activation(attnT[2][:, csl], tps,
                                         mybir.ActivationFunctionType.Copy)
                else:
                    nc.vector.tensor_copy(attnT[2][:, csl], tps)

            # cross-window O backlog: each pair's O-block (and, for a
            # window's last pair, its normalization) is queued and drained
            # `lag` pairs later - ACROSS window boundaries for heads 0-2,
            # so a window's trailing O's overlap the next window's S
            # stream.  Head-3 windows drain fully at their end because the
            # following window's repack thunks need the normalization.
            obl = []

            def emit_attention(h, thunks_for_qc):
                """thunks_for_qc(qc) -> list of emit callables injected into
                the PE stream spread across this q-chunk's pairs."""
                for qc in range(NQC):
                    thunks, pos = thunks_for_qc(qc)
                    inject_at = {}
                    if pos is not None:
                        for i, t in enumerate(thunks):
                            inject_at.setdefault(
                                pos[i] if i < len(pos) else NPAIR - 1,
                                []).append(t)
                    else:
                        step = max(NPAIR // max(len(thunks), 1), 1)
                        for i, t in enumerate(thunks):
                            inject_at.setdefault(
                                min(1 + i * step, NPAIR - 1), []).append(t)
                    qsl = slice(qc * QC, (qc + 1) * QC)
                    idx = h * NQC + qc
                    # flipped-O accumulator: [128 q, 4 qt x (96 d + sums)]
                    # packed per q-tile into one psum bank.  The bank holds
                    # FOUR interleaved accumulation chains; hardware supports
                    # only one OPEN start/stop group per bank at a time
                    # (later start=True resets the others), so the tile is
                    # zeroed up front and every matmul accumulates with
                    # start=False.
                    psof = ps_o.tile([128, 4 * HDV], f32_, tag="pso",
                                     name=f"psof{idx}")
                    nc.vector.memset(psof, 0.0)
                    eps = []

                    def emit_ss(p):
                        pss = ps_pair.tile([128, 1024], f32_, tag="pss",
                                           name=f"pss{idx}_{p}")
                        ep = expp.tile([128, 1024], bf, tag="exp",
                                       name=f"exp{idx}_{p}")
                        # exp split: Pool takes the FIRST cols (waits only
                        # on the j=0 matmul, so its q7-launch latency hides
                        # behind the j=1 matmul), ACT takes the rest.  Both
                        # cadences stay under the pair's PE work so PE is
                        # the limiter.  Head 3 (qc>0) keeps exp fully on ACT
                        # - its windows carry out-proj thunks (PE/pair well
                        # above 1038 ns) and Pool absorbs psum evictions.
                        # both S matmuls BEFORE either exp: an exp emitted
                        # between them reads this pss tile and Tile's
                        # conservative subtile tracking then serializes the
                        # second matmul behind it (false intra-tile WAR)
                        nc.tensor.matmul(
                            pss[:, 0:512],
                            lhsT=kT[h][:, (2 * p) * 128:(2 * p + 1) * 128],
                            rhs=qT[h][:, qsl],
                            start=True, stop=True)
                        nc.tensor.matmul(
                            pss[:, 512:1024],
                            lhsT=kT[h][:, (2 * p + 1) * 128:
                                       (2 * p + 2) * 128],
                            rhs=qT[h][:, qsl],
                            start=True, stop=True)
                        nc.scalar.activation(
                            ep, pss,
                            mybir.ActivationFunctionType.Exp,
                            scale=SCALING)
                        eps.append(ep)

                    def emit_o(p, ep, psof=psof, h=h):
                        # flipped: the exp tile is the (free) stationary
                        # load, the 97-col vaug slab streams -> 40 ns per
                        # matmul instead of 213, output lands [q, d]
                        for j in range(2):
                            kt = 2 * p + j
                            for qt in range(4):
                                nc.tensor.matmul(
                                    psof[:, qt * HDV:(qt + 1) * HDV],
                                    lhsT=ep[:, j * 512 + qt * 128:
                                            j * 512 + (qt + 1) * 128],
                                    rhs=vaug[kt][:, h * HDV:(h + 1) * HDV],
                                    start=False, stop=(kt == NKT - 1))

                    def emit_norm(psof=psof, h=h, qc=qc, idx=idx):
                        # sums live at col 96 of each q-tile group; the
                        # reciprocal is a per-partition scalar.  One
                        # contiguous single-column reciprocal per q-tile
                        # (the custom-DVE op mis-lowers strided APs).
                        rbq = rbp.tile([128, 4], f32_, tag="rbq",
                                       name=f"rbq{idx}")
                        for qt in range(4):
                            nc.vector.reciprocal_approx_fast(
                                out=rbq[:, qt:qt + 1],
                                in_=psof[:, qt * HDV + HD:qt * HDV + HD + 1])
                        for qt in range(4):
                            nc.vector.tensor_scalar_mul(
                                normed[qc][qt][:, h * HD:(h + 1) * HD],
                                psof[:, qt * HDV:qt * HDV + HD],
                                rbq[:, qt:qt + 1])

                    def make_o(p, last, ep):
                        def f():
                            emit_o(p, ep)
                            if last:
                                emit_norm()
                        return f

                    lag = 4
                    for p in range(NPAIR):
                        emit_ss(p)
                        for t in inject_at.get(p, ()):
                            t()
                        obl.append(make_o(p, p == NPAIR - 1, eps[-1]))
                        while len(obl) > lag:
                            obl.pop(0)()
                    while obl:
                        obl.pop(0)()

            # ---- emission schedule ----
            # consume each x seq-block as its DMA lands: head 0's k/q chunks
            # for block n, then the V chunks of block n
            for n in range(4):
                emit_kq_chunk(0, n, "k")
                emit_kq_chunk(0, n, "q")
                for kt in range(4 * n, 4 * n + 4):
                    emit_v_chunk(kt)

            def kq_thunks(hnext):
                def f(qc):
                    # 2 chunks per q-chunk, split into 3-matmul halves (4
                    # thunks of ~640 ns) so the PE filler spreads across the
                    # window instead of lumping at two pairs
                    items = list(kq_chunks(hnext))[2 * qc:2 * qc + 2]
                    out = []
                    for it in items:
                        for part in range(2):
                            out.append(
                                lambda it=it, part=part: emit_kq_part(
                                    it[1], it[2], it[0], part))
                    return out, None
                return f

            def rp01_thunks(qc):
                return [lambda qt=qt: emit_repack01(qc, qt)
                        for qt in range(4)]

            def out_thunks(qc):
                # head-3 window qc: close out q-chunk qc-1 (t2 repack + P/F
                # two-phase out-proj) and pre-repack t0/t1 of q-chunk qc+1
                # (they only need heads 0-2, normalized long ago).  The
                # last window also starts P(q0+0) with its psB on the pso
                # spare slot so the tail begins with a partial in flight.
                if qc == 0:
                    return (rp01_thunks(0) + rp01_thunks(1)
                            + rp01_thunks(2),
                            [0, 1, 1, 2, 3, 3, 4, 5, 5, 6, 7, 7])
                qp = qc - 1
                out = []
                for qt4 in range(0, 4, 2):
                    out.append(lambda qt=qt4: emit_repack2(qp, qt))
                    out.append(lambda qt=qt4 + 1: emit_repack2(qp, qt))
                    for qm in (4 * qp + qt4, 4 * qp + qt4 + 1):
                        out.append(
                            lambda qm=qm: emit_out_partial(qm, "win"))
                        out.append(lambda qm=qm: emit_out_finish(qm, True))
                pos = [0, 0, 1, 1, 2, 2, 3, 3, 4, 4, 5, 5]
                if qc == NQC - 2:
                    out += rp01_thunks(qc + 1)
                    pos += [6, 6, 7, 7]
                elif qc == NQC - 1:
                    out.append(lambda: emit_out_partial(4 * qc, "edge"))
                    pos += [7]
                return out, pos

            for h in range(HPC - 1):
                emit_attention(h, kq_thunks(h + 1))
            emit_attention(HPC - 1, out_thunks)
            # tail: only the t2 repacks and the finish halves remain; each
            # F(qm) frees the ps/pso slots its successor partial needs
            q0 = 4 * (NQC - 1)
            emit_repack2(NQC - 1, 0, True)
            emit_out_finish(q0)
            emit_out_partial(q0 + 1)
            emit_repack2(NQC - 1, 1, True)
            emit_out_finish(q0 + 1)
            emit_out_partial(q0 + 2)
            emit_repack2(NQC - 1, 2, True)
            emit_out_finish(q0 + 2)
            emit_out_partial(q0 + 3)
            emit_repack2(NQC - 1, 3, True)
            emit_out_finish(q0 + 3)

            if os.environ.get("KDEBUG"):
                dbg_attn = nc.dram_tensor(
                    "dbg_attn", [3 * 128, SEQ], bf,
                    kind="ExternalOutput").ap()
                for t_ in range(3):
                    nc.sync.dma_start(
                        out=dbg_attn[t_ * 128:(t_ + 1) * 128, :],
                        in_=attnT[t_])
                dbg_nm = nc.dram_tensor(
                    "dbg_nm", [NQC * 4 * 128, HPC * HD], bf,
                    kind="ExternalOutput").ap()
                for qc in range(NQC):
                    for qt in range(4):
                        r0 = (qc * 4 + qt) * 128
                        nc.sync.dma_start(out=dbg_nm[r0:r0 + 128, :],
                                          in_=normed[qc][qt])

    nc.compile()
    return nc


def _get_nc():
    if "nc" not in _NC_CACHE:
        _NC_CACHE["nc"] = _build_nc()
    return _NC_CACHE["nc"]


def _etile_pack(wT):
    """[768, n] (e on rows) -> [128, 6*n] bf16: e-tiles as column blocks so
    the whole operand loads as ONE [128, n] DMA."""
    n = wT.shape[1]
    a = wT.reshape(NE, 128, n).transpose(1, 0, 2)
    return np.ascontiguousarray(a.reshape(128, NE * n)).astype(BF16)


def _x_block_pack(x_b):
    """[2048, 768] x -> [128, 4 * 6 * 512] bf16, seq-block-major: block n
    holds e-tiles of sequence rows n*512..(n+1)*512 as column slabs."""
    a = x_b.reshape(4, 512, NE, 128)          # n, s, e, p
    a = a.transpose(3, 0, 2, 1)               # p, n, e, s
    return np.ascontiguousarray(a.reshape(128, 4 * NE * 512)).astype(BF16)


def _pad_headsT(w_rows):
    """[384, 768] head rows -> zero-pad head dim 96->128 -> transpose -> [768, 512]."""
    p = np.zeros((HPC * HDP, EMB), np.float32)
    p.reshape(HPC, HDP, EMB)[:, :HD] = w_rows.reshape(HPC, HD, EMB)
    return np.ascontiguousarray(p.T)


def _pad_bias(b_rows):
    """[384] head bias -> [128, HPC] padded/transposed for per-partition add."""
    p = np.zeros((HPC, HDP), np.float32)
    p[:, :HD] = b_rows.reshape(HPC, HD)
    return np.ascontiguousarray(p.T)


def kernel(x, Wq, bq, Wk, bk, Wv, bv, Wo, bo):
    x = np.asarray(x, np.float32)
    Wq, bq = np.asarray(Wq, np.float32), np.asarray(bq, np.float32)
    Wk, bk = np.asarray(Wk, np.float32), np.asarray(bk, np.float32)
    Wv, bv = np.asarray(Wv, np.float32), np.asarray(bv, np.float32)
    Wo, bo = np.asarray(Wo, np.float32), np.asarray(bo, np.float32)

    nc = _get_nc()

    in_maps = []
    for c in range(NCORES):
        b, hg = divmod(c, 2)
        hs = slice(hg * HPC * HD, (hg + 1) * HPC * HD)
        woT = Wo[:, hs].T  # [384, 768]
        wo_pack = np.ascontiguousarray(
            woT.reshape(3, 128, EMB).transpose(1, 0, 2).reshape(128, 3 * EMB))
        wq_et = _etile_pack(_pad_headsT(Wq[hs])).reshape(128, NE, HPC, HDP)
        wk_et = _etile_pack(_pad_headsT(Wk[hs])).reshape(128, NE, HPC, HDP)
        in_maps.append({
            "xtp": _x_block_pack(x[b]),
            "wqp0": np.ascontiguousarray(
                wq_et[:, :, 0].reshape(128, NE * HDP)),
            "wqpr": np.ascontiguousarray(
                wq_et[:, :, 1:].reshape(128, NE * 3 * HDP)),
            "wkp0": np.ascontiguousarray(
                wk_et[:, :, 0].reshape(128, NE * HDP)),
            "wkpr": np.ascontiguousarray(
                wk_et[:, :, 1:].reshape(128, NE * 3 * HDP)),
            "wvp": _etile_pack(np.ascontiguousarray(Wv[hs].T)),
            "wop": wo_pack.astype(BF16),
            "bqp": _pad_bias(bq[hs]),
            "bkp": _pad_bias(bk[hs]),
            "identp": np.ascontiguousarray(np.eye(128, dtype=np.float32))
            .astype(BF16),
        })

    global LAST_RESULT
    trace = bool(int(os.environ.get("KERNEL_TRACE", "0")))
    tmpdir = os.environ.get("KERNEL_TRACE_DIR") or None
    res = run_bass_kernel_spmd(nc, in_maps, list(range(NCORES)), trace=trace,
                               tmpdir=tmpdir)
    LAST_RESULT = res

    out = np.empty((B, SEQ, EMB), np.float32)
    for b in range(B):
        out[b] = res.results[2 * b]["outp"] + res.results[2 * b + 1]["outp"]
    # bv enters each head's output additively (sum of softmax weights is 1),
    # and bo is a plain add: both fold into one constant vector.
    out += Wo @ bv + bo
    return out



# revision 98
# speedup vs baseline: 1.0007x; 1.0007x over previous
"""Multi-head attention (B=4, S=2048, E=768, H=8, D=96) on 8 Trainium2 cores.

Sharding: core c -> (batch b = c//2, head-group hg = c%2 of 4 heads).
Each core computes Q/K/V projections for its 4 heads over the full sequence
of its batch, full attention for those heads, and a partial output
projection (row-split Wo).  The two cores of a batch produce partial
outputs that are summed on the host during unsharding (tensor-parallel
reduce).

On-chip layout notes:
  - All matmul operands are bf16 (1 cycle/row on PE; fp32 would be 4x; fp8
    DoubleRow would halve PE time but its ~3% per-element noise lands ~1:1
    in the output and busts the 2e-2 tolerance).
  - Scores are computed transposed, S^T[k, q] = K^T.T @ Q^T (the K/Q
    projections pad head dim 96->128 on the contraction partitions, which
    costs nothing).  exp(S) runs on ACT straight out of PSUM over a
    [128, 1024] pair of key tiles with the 1/sqrt(d) scale folded in.
    ACT's 1038 ns per pair is the cadence ceiling for bare windows.
  - The O = V^T exp(S) matmul is FLIPPED: the exp tile is the stationary
    operand (Ldweights is free) and a 97-column vaug slab (96 V dims + a
    ones column) streams, so each matmul costs 40 ns instead of 213 and
    the per-pair PE work drops from 852 to 749 ns.  The ones column makes
    the softmax denominator fall out at column 96 of each q-tile group of
    the [q, 4 x 97] accumulator, so normalization is a per-partition
    reciprocal + tensor_scalar - no broadcast machinery at all.
  - A PSUM bank supports only ONE open accumulation group at a time, so
    the flipped accumulator (4 interleaved chains in one bank) is zeroed
    up front and every matmul accumulates with start=False.
  - The normalized [q, d] output is transposed back to the [d, q] layout
    the output projection needs via 128x128 PE transposes (53 ns each)
    against a host-provided identity.  attnT chunks t0/t1 only contain
    heads 0-2, so they repack during head 3's own windows (the ACT-bound
    first window absorbs three q-chunks' worth as PE filler); only the t2
    chunk waits for head 3's normalization, which keeps the tail short.
  - O matmuls consume exps with a 4-pair lag so the ACT exp latency
    (~1.35 us from S matmul to usable exp) never stalls the PE.  Both S
    matmuls of a pair are emitted BEFORE the exp that reads them: Tile's
    conservative subtile tracking would otherwise serialize the second
    matmul behind the exp (false intra-tile WAR).
  - Inputs are host-packed so each operand is ONE [128, n] DMA.  x is
    packed seq-block-major; the first x half rides the Pool/SWDGE queue
    (independent of the HWDGE slot) so it overlaps the wk0 load; head 0's
    K/Q weight columns load before the other heads'.
  - PE stream order: head 0's K/Q projections and V chunks per x block,
    then 16 attention windows (4 heads x 4 q-chunks).  Heads 0-2 carry
    the next head's K/Q chunks as thunked PE filler; head 3's windows
    carry the previous q-chunk's t2 repack + two-phase output projection
    (P = t0/t1 partials, F = t2 close + DVE evictions + stores split
    across the SP/ACT/SWDGE queues).  PSUM rings: pss pairs (2x2 banks),
    flipped-O accumulators (2x1), projection/repack ring (2x1); every
    allocation is placed one eviction behind its ring slot's previous
    user.
"""

import os
import sys

sys.path.insert(0, "/opt/trn_rl_repo")

import numpy as np
import ml_dtypes

import concourse.bacc as bacc
import concourse.tile as tile
from concourse import mybir
from concourse.bass_utils import run_bass_kernel_spmd

BF16 = ml_dtypes.bfloat16

EMB = 768
HEADS = 8
HD = 96          # true head dim
HDP = 128        # padded head dim
SEQ = 2048
B = 4
NCORES = 8
HPC = 4          # heads per core
SCALING = HD ** -0.5
QC = 512         # query chunk per attention inner loop
NQC = SEQ // QC
NKT = SEQ // 128  # 16 key tiles
NPAIR = NKT // 2
NE = EMB // 128   # 6 e_in tiles

_NC_CACHE = {}
LAST_RESULT = None  # BassKernelResults of the most recent run (for test.py)


def _build_nc():
    f32 = mybir.dt.float32
    bf = mybir.dt.bfloat16

    nc = bacc.Bacc(trn_type="TRN2", target_bir_lowering=False, debug=False,
                   num_devices=NCORES)

    # All operands host-packed into [128, n] so each loads as ONE DMA.
    xtp = nc.dram_tensor("xtp", [128, NE * SEQ], bf, kind="ExternalInput").ap()
    # K/Q weights split: head 0's columns load first (0.19MB) so head 0's
    # projections - which gate the whole pipeline - start ~5 us earlier
    wqp0 = nc.dram_tensor("wqp0", [128, NE * HDP], bf,
                          kind="ExternalInput").ap()
    wqpr = nc.dram_tensor("wqpr", [128, NE * 3 * HDP], bf,
                          kind="ExternalInput").ap()
    wkp0 = nc.dram_tensor("wkp0", [128, NE * HDP], bf,
                          kind="ExternalInput").ap()
    wkpr = nc.dram_tensor("wkpr", [128, NE * 3 * HDP], bf,
                          kind="ExternalInput").ap()
    wvp = nc.dram_tensor("wvp", [128, NE * HPC * HD], bf,
                         kind="ExternalInput").ap()
    wop = nc.dram_tensor("wop", [128, 3 * EMB], bf, kind="ExternalInput").ap()
    bqp = nc.dram_tensor("bqp", [128, HPC], f32, kind="ExternalInput").ap()
    bkp = nc.dram_tensor("bkp", [128, HPC], f32, kind="ExternalInput").ap()
    identp = nc.dram_tensor("identp", [128, 128], bf,
                            kind="ExternalInput").ap()
    outp = nc.dram_tensor("outp", [SEQ, EMB], f32, kind="ExternalOutput").ap()

    with tile.TileContext(nc) as tc:
        with (
            tc.tile_pool(name="const", bufs=1) as constp,
            tc.tile_pool(name="big", bufs=1) as bigp,
            tc.tile_pool(name="expp", bufs=6) as expp,
            tc.tile_pool(name="rbp", bufs=4) as rbp,
            tc.tile_pool(name="normp", bufs=1) as normp,
            tc.tile_pool(name="outsb", bufs=12) as outsb,
            tc.tile_pool(name="ps_proj", bufs=2, space="PSUM") as ps_proj,
            tc.tile_pool(name="ps_o", bufs=2, space="PSUM") as ps_o,
            tc.tile_pool(name="ps_pair", bufs=2, space="PSUM") as ps_pair,
        ):
            # ---- loads. x is packed seq-block-major ([128, 6e x 512] per
            # 512-sequence block) so the first K/Q chunk only needs wk + one
            # 0.75MB block; wk + block 0/1 load first, k/q chunks of head 0
            # then pipeline behind the remaining block DMAs. ----
            XB = NE * 512  # 3072 cols per seq block
            wk0_sb = constp.tile([128, NE * HDP], bf, name="wk0_sb")
            nc.sync.dma_start(out=wk0_sb, in_=wkp0)
            xtb = [bigp.tile([128, XB], bf, name=f"xtb{n}") for n in range(4)]
            # the first x half rides the Pool/SWDGE queue: its descriptor
            # path is independent of the HWDGE slot the weight loads use,
            # so it overlaps wk0 end-to-end
            nc.gpsimd.dma_start(out=xtb[0][:, 0:XB // 2],
                                in_=xtp[:, 0:XB // 2])
            nc.sync.dma_start(out=xtb[0][:, XB // 2:XB],
                              in_=xtp[:, XB // 2:XB])
            # biases next: tiny, but they gate the K/Q psum evictions
            bq_sb = constp.tile([128, HPC], f32, name="bq_sb")
            nc.sync.dma_start(out=bq_sb, in_=bqp)
            bk_sb = constp.tile([128, HPC], f32, name="bk_sb")
            nc.sync.dma_start(out=bk_sb, in_=bkp)
            wq0_sb = constp.tile([128, NE * HDP], bf, name="wq0_sb")
            nc.sync.dma_start(out=wq0_sb, in_=wqp0)
            wv_sb = constp.tile([128, NE * HPC * HD], bf, name="wv_sb")
            nc.sync.dma_start(out=wv_sb, in_=wvp)
            nc.sync.dma_start(out=xtb[1], in_=xtp[:, XB:2 * XB])
            wkr_sb = constp.tile([128, NE * 3 * HDP], bf, name="wkr_sb")
            nc.sync.dma_start(out=wkr_sb, in_=wkpr)
            wqr_sb = constp.tile([128, NE * 3 * HDP], bf, name="wqr_sb")
            nc.sync.dma_start(out=wqr_sb, in_=wqpr)
            nc.sync.dma_start(out=xtb[2], in_=xtp[:, 2 * XB:3 * XB])
            nc.sync.dma_start(out=xtb[3], in_=xtp[:, 3 * XB:4 * XB])
            wo_sb = constp.tile([128, 3 * EMB], bf, name="wo_sb")
            nc.sync.dma_start(out=wo_sb, in_=wop)
            # identity for the PE repack transposes (first needed in head
            # 3's phase, so it loads last)
            ident_sb = constp.tile([128, 128], bf, name="ident_sb")
            nc.sync.dma_start(out=ident_sb, in_=identp)

            def wk_eh(e, h):
                if h == 0:
                    return wk0_sb[:, e * HDP:(e + 1) * HDP]
                return wkr_sb[:, (e * 3 + h - 1) * HDP:(e * 3 + h) * HDP]

            def wq_eh(e, h):
                if h == 0:
                    return wq0_sb[:, e * HDP:(e + 1) * HDP]
                return wqr_sb[:, (e * 3 + h - 1) * HDP:(e * 3 + h) * HDP]

            def wv_e(e):
                return wv_sb[:, e * HPC * HD:(e + 1) * HPC * HD]

            def wo_t(t_):
                return wo_sb[:, t_ * EMB:(t_ + 1) * EMB]

            # ---- persistent intermediates ----
            # vaug: per key tile, 4 heads x (96 v-cols + a ones col).  The
            # ones col makes row^T @ vaug yield the softmax denominator in
            # the same accumulator (col 96 of each head group).  No pad
            # cols: vaug is the MOVING operand of the flipped O matmul, so
            # narrower means cheaper (97 vs 128 cycles).
            HDV = HD + 1
            vaug = []
            for kt in range(NKT):
                t = bigp.tile([128, HPC * HDV], bf, name=f"vaug{kt}")
                ones_cols = t.rearrange("p (h c) -> p h c",
                                        h=HPC)[:, :, HD:HD + 1]
                nc.gpsimd.memset(ones_cols, 1.0)
                vaug.append(t)
            qT = [bigp.tile([128, SEQ], bf, name=f"qT{h}") for h in range(HPC)]
            kT = [bigp.tile([128, SEQ], bf, name=f"kT{h}") for h in range(HPC)]
            # packed attention output, [384 rows = 3 tiles x 128, seq],
            # written by the h3-phase repack transposes
            attnT = [bigp.tile([128, SEQ], bf, name=f"attnT{t_}")
                     for t_ in range(3)]
            # normalized O in [q, head-dims] layout, per (q-chunk, q-tile):
            # written per head as its window completes, transposed into
            # attnT once all four heads are in
            normed = [[normp.tile([128, HPC * HD], bf, name=f"nm{qc}_{qt}")
                       for qt in range(4)] for qc in range(NQC)]

            f32_ = f32

            # ---- projection emit helpers ----
            def emit_v_chunk(kt):
                psv = ps_proj.tile([128, 512], f32_, tag="ps",
                                   name=f"psv{kt}")
                blk, off = divmod(kt, 4)
                for e in range(NE):
                    nc.tensor.matmul(psv[:, 0:HPC * HD],
                                     lhsT=xtb[blk][:, e * 512 + off * 128:
                                                   e * 512 + off * 128 + 128],
                                     rhs=wv_e(e),
                                     start=(e == 0), stop=(e == NE - 1))
                for hh in range(HPC):
                    nc.vector.tensor_copy(
                        vaug[kt][:, hh * HDV:hh * HDV + HD],
                        psv[:, hh * HD:(hh + 1) * HD])

            kq_ps = {}

            def emit_kq_part(h, n, which, part):
                """One half of a K/Q chunk (3 of 6 e-matmuls).  Thunking
                chunks at this granularity keeps the per-pair PE load above
                the ACT exp cadence, so the S stream never outruns ACT into
                the 2-slot pss ring."""
                key = (h, n, which)
                w_eh, dst, b_sb = ((wk_eh, kT, bk_sb) if which == "k"
                                   else (wq_eh, qT, bq_sb))
                if part == 0:
                    kq_ps[key] = ps_proj.tile([128, 512], f32_, tag="ps",
                                              name=f"ps{which}{h}_{n}")
                ps = kq_ps[key]
                for e in (3 * part, 3 * part + 1, 3 * part + 2):
                    nc.tensor.matmul(ps,
                                     lhsT=w_eh(e, h),
                                     rhs=xtb[n][:, e * 512:(e + 1) * 512],
                                     start=(e == 0), stop=(e == NE - 1))
                if part == 1:
                    nsl = slice(n * 512, (n + 1) * 512)
                    nc.vector.tensor_scalar_add(dst[h][:, nsl],
                                                kq_ps.pop(key),
                                                b_sb[:, h:h + 1])

            def emit_kq_chunk(h, n, which):
                for part in range(2):
                    emit_kq_part(h, n, which, part)

            def kq_chunks(h):
                for n in range(4):
                    yield ("k", h, n)
                    yield ("q", h, n)

            # ---- output projection chunk (one 128-row q tile) ----
            # Split across two 1-bank psums so it can borrow ps_proj slots;
            # PSUM->SBUF copies go on DVE (ACT is busy with exp here).
            # Tail out-proj chunks, two-phase: head 3's rows live only in
            # packed attnT tile 2, so the t0/t1 matmuls are independent of
            # the final normalization and run DURING its chain; only the t2
            # matmul (accumulation close) waits for the per-qm norm slice.
            # Psums borrow the attention rings (free by then).
            tail_ps = {}

            def emit_out_partial(qm, mode="tail"):
                qsl = slice(qm * 128, (qm + 1) * 128)
                if mode == "win":
                    psA = ps_proj.tile([128, 512], f32_, tag="ps",
                                       name=f"poA{qm}")
                    psB = ps_proj.tile([128, 512], f32_, tag="ps",
                                       name=f"poB{qm}")
                elif mode == "edge":
                    # last-window partial: psA on the ps ring, psB on the
                    # pso ring's spare slot (the previous psof freed once
                    # its normalization drained) so only ONE ps slot is
                    # held across the window/tail boundary
                    psA = ps_proj.tile([128, 512], f32_, tag="ps",
                                       name=f"poA{qm}")
                    psB = ps_o.tile([128, 512], f32_, tag="pso",
                                    name=f"poB{qm}")
                else:
                    # tail: both halves on the pss ring (free then)
                    psA = ps_pair.tile([128, 512], f32_, tag="pss",
                                       name=f"poA{qm}")
                    psB = ps_pair.tile([128, 512], f32_, tag="pss",
                                       name=f"poB{qm}")
                tail_ps[qm] = (psA, psB)
                for t in range(2):
                    nc.tensor.matmul(psA,
                                     lhsT=attnT[t][:, qsl],
                                     rhs=wo_t(t)[:, 0:512],
                                     start=(t == 0), stop=False)
                    nc.tensor.matmul(psB[:, 0:256],
                                     lhsT=attnT[t][:, qsl],
                                     rhs=wo_t(t)[:, 512:768],
                                     start=(t == 0), stop=False)

            def emit_out_finish(qm, windowed=False):
                qsl = slice(qm * 128, (qm + 1) * 128)
                psA, psB = tail_ps.pop(qm)
                nc.tensor.matmul(psA, lhsT=attnT[2][:, qsl],
                                 rhs=wo_t(2)[:, 0:512],
                                 start=False, stop=True)
                nc.tensor.matmul(psB[:, 0:256], lhsT=attnT[2][:, qsl],
                                 rhs=wo_t(2)[:, 512:768],
                                 start=False, stop=True)
                out_sb = outsb.tile([128, EMB], f32_, tag="osb",
                                    name=f"osb{qm}")
                # evictions on DVE in windows (ACT is exp-saturated there;
                # GPSIMD cannot touch PSUM on real hw).  In the tail, psB
                # rides ACT end-to-end (evict + store) so the last stores
                # split across the SP and ACT queues instead of
                # serializing on SP's 650 ns issue cost.
                nc.vector.tensor_copy(out_sb[:, 0:512], psA)
                if windowed:
                    nc.sync.dma_start(
                        out=outp[qm * 128:(qm + 1) * 128, 0:512],
                        in_=out_sb[:, 0:512])
                else:
                    eng = nc.gpsimd if qm % 2 == 0 else nc.sync
                    eng.dma_start(
                        out=outp[qm * 128:(qm + 1) * 128, 0:512],
                        in_=out_sb[:, 0:512])
                if windowed:
                    nc.vector.tensor_copy(out_sb[:, 512:768], psB[:, 0:256])
                    nc.sync.dma_start(
                        out=outp[qm * 128:(qm + 1) * 128, 512:768],
                        in_=out_sb[:, 512:768])
                else:
                    # tail: psB evict on DVE; store rotated across the
                    # ACT and Pool/SWDGE queues so the four final stores
                    # don't serialize on the single HWDGE slot
                    nc.vector.tensor_copy(out_sb[:, 512:768], psB[:, 0:256])
                    eng = nc.scalar if qm % 2 == 0 else nc.gpsimd
                    eng.dma_start(
                        out=outp[qm * 128:(qm + 1) * 128, 512:768],
                        in_=out_sb[:, 512:768])

            # ---- attention emit (with interleaved PE filler work) ----
            # repack: transpose normed[qc][qt] ([128 q, 384 packed head
            # dims]) into attnT via 128x128 PE transposes + DVE evictions.
            # Chunks t0/t1 cover only heads 0-2, so they repack as soon as
            # head 2's normalization lands - during head 3's own windows -
            # and the out-proj partials (which read t0/t1) can then run
            # inside the last window.  Only the t2 chunk (heads 2+3) waits
            # for head 3's normalization.
            def emit_repack01(qc, qt):
                tps = ps_proj.tile([128, 2 * 128], bf, tag="ps",
                                   name=f"tp01_{qc}_{qt}")
                for t_ in range(2):
                    nc.tensor.transpose(
                        tps[:, t_ * 128:(t_ + 1) * 128],
                        normed[qc][qt][:, t_ * 128:(t_ + 1) * 128],
                        ident_sb)
                csl = slice(qc * QC + qt * 128, qc * QC + (qt + 1) * 128)
                for t_ in range(2):
                    nc.vector.tensor_copy(attnT[t_][:, csl],
                                          tps[:, t_ * 128:(t_ + 1) * 128])

            def emit_repack2(qc, qt, tail=False):
                tps = ps_proj.tile([128, 128], bf, tag="ps",
                                   name=f"tp2_{qc}_{qt}")
                nc.tensor.transpose(tps, normed[qc][qt][:, 256:384],
                                    ident_sb)
                csl = slice(qc * QC + qt * 128, qc * QC + (qt + 1) * 128)
                if tail:
                    # keep the tail's DVE queue clear for the out evictions
                    nc.scalar.activation(attnT[2][:, csl], tps,
                                         mybir.ActivationFunctionType.Copy)
                else:
                    nc.vector.tensor_copy(attnT[2][:, csl], tps)

            # cross-window O backlog: each pair's O-block (and, for a
            # window's last pair, its normalization) is queued and drained
            # `lag` pairs later - ACROSS window boundaries for heads 0-2,
            # so a window's trailing O's overlap the next window's S
            # stream.  Head-3 windows drain fully at their end because the
            # following window's repack thunks need the normalization.
            obl = []

            def emit_attention(h, thunks_for_qc):
                """thunks_for_qc(qc) -> list of emit callables injected into
                the PE stream spread across this q-chunk's pairs."""
                for qc in range(NQC):
                    thunks, pos = thunks_for_qc(qc)
                    inject_at = {}
                    if pos is not None:
                        for i, t in enumerate(thunks):
                            inject_at.setdefault(
                                pos[i] if i < len(pos) else NPAIR - 1,
                                []).append(t)
                    else:
                        for i, t in enumerate(thunks):
                            inject_at.setdefault(
                                min(2 * i, NPAIR - 1), []).append(t)
                    qsl = slice(qc * QC, (qc + 1) * QC)
                    idx = h * NQC + qc
                    # flipped-O accumulator: [128 q, 4 qt x (96 d + sums)]
                    # packed per q-tile into one psum bank.  The bank holds
                    # FOUR interleaved accumulation chains; hardware supports
                    # only one OPEN start/stop group per bank at a time
                    # (later start=True resets the others), so the tile is
                    # zeroed up front and every matmul accumulates with
                    # start=False.
                    psof = ps_o.tile([128, 4 * HDV], f32_, tag="pso",
                                     name=f"psof{idx}")
                    nc.vector.memset(psof, 0.0)
                    eps = []

                    def emit_ss(p):
                        pss = ps_pair.tile([128, 1024], f32_, tag="pss",
                                           name=f"pss{idx}_{p}")
                        ep = expp.tile([128, 1024], bf, tag="exp",
                                       name=f"exp{idx}_{p}")
                        # exp split: Pool takes the FIRST cols (waits only
                        # on the j=0 matmul, so its q7-launch latency hides
                        # behind the j=1 matmul), ACT takes the rest.  Both
                        # cadences stay under the pair's PE work so PE is
                        # the limiter.  Head 3 (qc>0) keeps exp fully on ACT
                        # - its windows carry out-proj thunks (PE/pair well
                        # above 1038 ns) and Pool absorbs psum evictions.
                        # both S matmuls BEFORE either exp: an exp emitted
                        # between them reads this pss tile and Tile's
                        # conservative subtile tracking then serializes the
                        # second matmul behind it (false intra-tile WAR)
                        nc.tensor.matmul(
                            pss[:, 0:512],
                            lhsT=kT[h][:, (2 * p) * 128:(2 * p + 1) * 128],
                            rhs=qT[h][:, qsl],
                            start=True, stop=True)
                        nc.tensor.matmul(
                            pss[:, 512:1024],
                            lhsT=kT[h][:, (2 * p + 1) * 128:
                                       (2 * p + 2) * 128],
                            rhs=qT[h][:, qsl],
                            start=True, stop=True)
                        nc.scalar.activation(
                            ep, pss,
                            mybir.ActivationFunctionType.Exp,
                            scale=SCALING)
                        eps.append(ep)

                    def emit_o(p, ep, psof=psof, h=h):
                        # flipped: the exp tile is the (free) stationary
                        # load, the 97-col vaug slab streams -> 40 ns per
                        # matmul instead of 213, output lands [q, d]
                        for j in range(2):
                            kt = 2 * p + j
                            for qt in range(4):
                                nc.tensor.matmul(
                                    psof[:, qt * HDV:(qt + 1) * HDV],
                                    lhsT=ep[:, j * 512 + qt * 128:
                                            j * 512 + (qt + 1) * 128],
                                    rhs=vaug[kt][:, h * HDV:(h + 1) * HDV],
                                    start=False, stop=(kt == NKT - 1))

                    def emit_norm(psof=psof, h=h, qc=qc, idx=idx):
                        # sums live at col 96 of each q-tile group; the
                        # reciprocal is a per-partition scalar.  One
                        # contiguous single-column reciprocal per q-tile
                        # (the custom-DVE op mis-lowers strided APs).
                        rbq = rbp.tile([128, 4], f32_, tag="rbq",
                                       name=f"rbq{idx}")
                        for qt in range(4):
                            nc.vector.reciprocal_approx_fast(
                                out=rbq[:, qt:qt + 1],
                                in_=psof[:, qt * HDV + HD:qt * HDV + HD + 1])
                        for qt in range(4):
                            nc.vector.tensor_scalar_mul(
                                normed[qc][qt][:, h * HD:(h + 1) * HD],
                                psof[:, qt * HDV:qt * HDV + HD],
                                rbq[:, qt:qt + 1])

                    def make_o(p, last, ep):
                        def f():
                            emit_o(p, ep)
                            if last:
                                emit_norm()
                        return f

                    lag = 4
                    for p in range(NPAIR):
                        emit_ss(p)
                        for t in inject_at.get(p, ()):
                            t()
                        obl.append(make_o(p, p == NPAIR - 1, eps[-1]))
                        while len(obl) > lag:
                            obl.pop(0)()
                    while obl:
                        obl.pop(0)()

            # ---- emission schedule ----
            # consume each x seq-block as its DMA lands: head 0's k/q chunks
            # for block n, then the V chunks of block n
            for n in range(4):
                emit_kq_chunk(0, n, "k")
                emit_kq_chunk(0, n, "q")
                for kt in range(4 * n, 4 * n + 4):
                    emit_v_chunk(kt)

            def kq_thunks(hnext):
                def f(qc):
                    # 2 chunks per q-chunk, split into 3-matmul halves (4
                    # thunks of ~640 ns) so the PE filler spreads across the
                    # window instead of lumping at two pairs
                    items = list(kq_chunks(hnext))[2 * qc:2 * qc + 2]
                    out = []
                    for it in items:
                        for part in range(2):
                            out.append(
                                lambda it=it, part=part: emit_kq_part(
                                    it[1], it[2], it[0], part))
                    return out, None
                return f

            def rp01_thunks(qc):
                return [lambda qt=qt: emit_repack01(qc, qt)
                        for qt in range(4)]

            def out_thunks(qc):
                # head-3 window qc: close out q-chunk qc-1 (t2 repack + P/F
                # two-phase out-proj) and pre-repack t0/t1 of q-chunk qc+1
                # (they only need heads 0-2, normalized long ago).  The
                # last window also starts P(q0+0) with its psB on the pso
                # spare slot so the tail begins with a partial in flight.
                if qc == 0:
                    return (rp01_thunks(0) + rp01_thunks(1)
                            + rp01_thunks(2),
                            [0, 1, 1, 2, 3, 3, 4, 5, 5, 6, 7, 7])
                qp = qc - 1
                out = []
                for qt4 in range(0, 4, 2):
                    out.append(lambda qt=qt4: emit_repack2(qp, qt))
                    out.append(lambda qt=qt4 + 1: emit_repack2(qp, qt))
                    for qm in (4 * qp + qt4, 4 * qp + qt4 + 1):
                        out.append(
                            lambda qm=qm: emit_out_partial(qm, "win"))
                        out.append(lambda qm=qm: emit_out_finish(qm, True))
                pos = [0, 0, 1, 1, 2, 2, 3, 3, 4, 4, 5, 5]
                if qc == NQC - 2:
                    out += rp01_thunks(qc + 1)
                    pos += [6, 6, 7, 7]
                elif qc == NQC - 1:
                    out.append(lambda: emit_out_partial(4 * qc, "edge"))
                    pos += [7]
                return out, pos

            for h in range(HPC - 1):
                emit_attention(h, kq_thunks(h + 1))
            emit_attention(HPC - 1, out_thunks)
            # tail: only the t2 repacks and the finish halves remain; each
            # F(qm) frees the ps/pso slots its successor partial needs
            q0 = 4 * (NQC - 1)
            emit_repack2(NQC - 1, 0, True)
            emit_out_finish(q0)
            emit_out_partial(q0 + 1)
            emit_repack2(NQC - 1, 1, True)
            emit_out_finish(q0 + 1)
            emit_out_partial(q0 + 2)
            emit_repack2(NQC - 1, 2, True)
            emit_out_finish(q0 + 2)
            emit_out_partial(q0 + 3)
            emit_repack2(NQC - 1, 3, True)
            emit_out_finish(q0 + 3)

            if os.environ.get("KDEBUG"):
                dbg_attn = nc.dram_tensor(
                    "dbg_attn", [3 * 128, SEQ], bf,
                    kind="ExternalOutput").ap()
                for t_ in range(3):
                    nc.sync.dma_start(
                        out=dbg_attn[t_ * 128:(t_ + 1) * 128, :],
                        in_=attnT[t_])
                dbg_nm = nc.dram_tensor(
                    "dbg_nm", [NQC * 4 * 128, HPC * HD], bf,
                    kind="ExternalOutput").ap()
                for qc in range(NQC):
                    for qt in range(4):
                        r0 = (qc * 4 + qt) * 128
                        nc.sync.dma_start(out=dbg_nm[r0:r0 + 128, :],
                                          in_=normed[qc][qt])

    nc.compile()
    return nc


def _get_nc():
    if "nc" not in _NC_CACHE:
        _NC_CACHE["nc"] = _build_nc()
    return _NC_CACHE["nc"]


def _etile_pack(wT):
    """[768, n] (e on rows) -> [128, 6*n] bf16: e-tiles as column blocks so
    the whole operand loads as ONE [128, n] DMA."""
    n = wT.shape[1]
    a = wT.reshape(NE, 128, n).transpose(1, 0, 2)
    return np.ascontiguousarray(a.reshape(128, NE * n)).astype(BF16)


def _x_block_pack(x_b):
    """[2048, 768] x -> [128, 4 * 6 * 512] bf16, seq-block-major: block n
    holds e-tiles of sequence rows n*512..(n+1)*512 as column slabs."""
    a = x_b.reshape(4, 512, NE, 128)          # n, s, e, p
    a = a.transpose(3, 0, 2, 1)               # p, n, e, s
    return np.ascontiguousarray(a.reshape(128, 4 * NE * 512)).astype(BF16)


def _pad_headsT(w_rows):
    """[384, 768] head rows -> zero-pad head dim 96->128 -> transpose -> [768, 512]."""
    p = np.zeros((HPC * HDP, EMB), np.float32)
    p.reshape(HPC, HDP, EMB)[:, :HD] = w_rows.reshape(HPC, HD, EMB)
    return np.ascontiguousarray(p.T)


def _pad_bias(b_rows):
    """[384] head bias -> [128, HPC] padded/transposed for per-partition add."""
    p = np.zeros((HPC, HDP), np.float32)
    p[:, :HD] = b_rows.reshape(HPC, HD)
    return np.ascontiguousarray(p.T)


def kernel(x, Wq, bq, Wk, bk, Wv, bv, Wo, bo):
    x = np.asarray(x, np.float32)
    Wq, bq = np.asarray(Wq, np.float32), np.asarray(bq, np.float32)
    Wk, bk = np.asarray(Wk, np.float32), np.asarray(bk, np.float32)
    Wv, bv = np.asarray(Wv, np.float32), np.asarray(bv, np.float32)
    Wo, bo = np.asarray(Wo, np.float32), np.asarray(bo, np.float32)

    nc = _get_nc()

    in_maps = []
    for c in range(NCORES):
        b, hg = divmod(c, 2)
        hs = slice(hg * HPC * HD, (hg + 1) * HPC * HD)
        woT = Wo[:, hs].T  # [384, 768]
        wo_pack = np.ascontiguousarray(
            woT.reshape(3, 128, EMB).transpose(1, 0, 2).reshape(128, 3 * EMB))
        wq_et = _etile_pack(_pad_headsT(Wq[hs])).reshape(128, NE, HPC, HDP)
        wk_et = _etile_pack(_pad_headsT(Wk[hs])).reshape(128, NE, HPC, HDP)
        in_maps.append({
            "xtp": _x_block_pack(x[b]),
            "wqp0": np.ascontiguousarray(
                wq_et[:, :, 0].reshape(128, NE * HDP)),
            "wqpr": np.ascontiguousarray(
                wq_et[:, :, 1:].reshape(128, NE * 3 * HDP)),
            "wkp0": np.ascontiguousarray(
                wk_et[:, :, 0].reshape(128, NE * HDP)),
            "wkpr": np.ascontiguousarray(
                wk_et[:, :, 1:].reshape(128, NE * 3 * HDP)),
            "wvp": _etile_pack(np.ascontiguousarray(Wv[hs].T)),
            "wop": wo_pack.astype(BF16),
            "bqp": _pad_bias(bq[hs]),
            "bkp": _pad_bias(bk[hs]),
            "identp": np.ascontiguousarray(np.eye(128, dtype=np.float32))
            .astype(BF16),
        })

    global LAST_RESULT
    trace = bool(int(os.environ.get("KERNEL_TRACE", "0")))
    tmpdir = os.environ.get("KERNEL_TRACE_DIR") or None
    res = run_bass_kernel_spmd(nc, in_maps, list(range(NCORES)), trace=trace,
                               tmpdir=tmpdir)
    LAST_RESULT = res

    out = np.empty((B, SEQ, EMB), np.float32)
    for b in range(B):
        out[b] = res.results[2 * b]["outp"] + res.results[2 * b + 1]["outp"]
    # bv enters each head's output additively (sum of softmax weights is 1),
    # and bo is a plain add: both fold into one constant vector.
    out += Wo @ bv + bo
    return out



# revision 111
# speedup vs baseline: 1.0013x; 1.0007x over previous
"""Multi-head attention (B=4, S=2048, E=768, H=8, D=96) on 8 Trainium2 cores.

Sharding: core c -> (batch b = c//2, head-group hg = c%2 of 4 heads).
Each core computes Q/K/V projections for its 4 heads over the full sequence
of its batch, full attention for those heads, and a partial output
projection (row-split Wo).  The two cores of a batch produce partial
outputs that are summed on the host during unsharding (tensor-parallel
reduce).

On-chip layout notes:
  - All matmul operands are bf16 (1 cycle/row on PE; fp32 would be 4x; fp8
    DoubleRow would halve PE time but its ~3% per-element noise lands ~1:1
    in the output and busts the 2e-2 tolerance).
  - Scores are computed transposed, S^T[k, q] = K^T.T @ Q^T (the K/Q
    projections pad head dim 96->128 on the contraction partitions, which
    costs nothing).  exp(S) runs on ACT straight out of PSUM over a
    [128, 1024] pair of key tiles with the 1/sqrt(d) scale folded in.
    ACT's 1038 ns per pair is the cadence ceiling for bare windows.
  - The O = V^T exp(S) matmul is FLIPPED: the exp tile is the stationary
    operand (Ldweights is free) and a 97-column vaug slab (96 V dims + a
    ones column) streams, so each matmul costs 40 ns instead of 213 and
    the per-pair PE work drops from 852 to 749 ns.  The ones column makes
    the softmax denominator fall out at column 96 of each q-tile group of
    the [q, 4 x 97] accumulator, so normalization is a per-partition
    reciprocal + tensor_scalar - no broadcast machinery at all.
  - A PSUM bank supports only ONE open accumulation group at a time, so
    the flipped accumulator (4 interleaved chains in one bank) is zeroed
    up front and every matmul accumulates with start=False.
  - The normalized [q, d] output is transposed back to the [d, q] layout
    the output projection needs via 128x128 PE transposes (53 ns each)
    against a host-provided identity.  attnT chunks t0/t1 only contain
    heads 0-2, so they repack during head 3's own windows (the ACT-bound
    first window absorbs three q-chunks' worth as PE filler); only the t2
    chunk waits for head 3's normalization, which keeps the tail short.
  - O matmuls consume exps with a 4-pair lag so the ACT exp latency
    (~1.35 us from S matmul to usable exp) never stalls the PE.  Both S
    matmuls of a pair are emitted BEFORE the exp that reads them: Tile's
    conservative subtile tracking would otherwise serialize the second
    matmul behind the exp (false intra-tile WAR).
  - Inputs are host-packed so each operand is ONE [128, n] DMA.  x is
    packed seq-block-major; the first x half rides the Pool/SWDGE queue
    (independent of the HWDGE slot) so it overlaps the wk0 load; head 0's
    K/Q weight columns load before the other heads'.
  - PE stream order: head 0's K/Q projections and V chunks per x block,
    then 16 attention windows (4 heads x 4 q-chunks).  Heads 0-2 carry
    the next head's K/Q chunks as thunked PE filler; head 3's windows
    carry the previous q-chunk's t2 repack + two-phase output projection
    (P = t0/t1 partials, F = t2 close + DVE evictions + stores split
    across the SP/ACT/SWDGE queues).  PSUM rings: pss pairs (2x2 banks),
    flipped-O accumulators (2x1), projection/repack ring (2x1); every
    allocation is placed one eviction behind its ring slot's previous
    user.
"""

import os
import sys

sys.path.insert(0, "/opt/trn_rl_repo")

import numpy as np
import ml_dtypes

import concourse.bacc as bacc
import concourse.tile as tile
from concourse import mybir
from concourse.bass_utils import run_bass_kernel_spmd

BF16 = ml_dtypes.bfloat16

EMB = 768
HEADS = 8
HD = 96          # true head dim
HDP = 128        # padded head dim
SEQ = 2048
B = 4
NCORES = 8
HPC = 4          # heads per core
SCALING = HD ** -0.5
QC = 512         # query chunk per attention inner loop
NQC = SEQ // QC
NKT = SEQ // 128  # 16 key tiles
NPAIR = NKT // 2
NE = EMB // 128   # 6 e_in tiles

_NC_CACHE = {}
LAST_RESULT = None  # BassKernelResults of the most recent run (for test.py)


def _build_nc():
    f32 = mybir.dt.float32
    bf = mybir.dt.bfloat16

    nc = bacc.Bacc(trn_type="TRN2", target_bir_lowering=False, debug=False,
                   num_devices=NCORES)

    # All operands host-packed into [128, n] so each loads as ONE DMA.
    xtp = nc.dram_tensor("xtp", [128, NE * SEQ], bf, kind="ExternalInput").ap()
    # K/Q weights split: head 0's columns load first (0.19MB) so head 0's
    # projections - which gate the whole pipeline - start ~5 us earlier
    wqp0 = nc.dram_tensor("wqp0", [128, NE * HDP], bf,
                          kind="ExternalInput").ap()
    wqpr = nc.dram_tensor("wqpr", [128, NE * 3 * HDP], bf,
                          kind="ExternalInput").ap()
    wkp0 = nc.dram_tensor("wkp0", [128, NE * HDP], bf,
                          kind="ExternalInput").ap()
    wkpr = nc.dram_tensor("wkpr", [128, NE * 3 * HDP], bf,
                          kind="ExternalInput").ap()
    wvp = nc.dram_tensor("wvp", [128, NE * HPC * HD], bf,
                         kind="ExternalInput").ap()
    wop = nc.dram_tensor("wop", [128, 3 * EMB], bf, kind="ExternalInput").ap()
    bqp = nc.dram_tensor("bqp", [128, HPC], f32, kind="ExternalInput").ap()
    bkp = nc.dram_tensor("bkp", [128, HPC], f32, kind="ExternalInput").ap()
    identp = nc.dram_tensor("identp", [128, 128], bf,
                            kind="ExternalInput").ap()
    outp = nc.dram_tensor("outp", [SEQ, EMB], f32, kind="ExternalOutput").ap()

    with tile.TileContext(nc) as tc:
        with (
            tc.tile_pool(name="const", bufs=1) as constp,
            tc.tile_pool(name="big", bufs=1) as bigp,
            tc.tile_pool(name="expp", bufs=6) as expp,
            tc.tile_pool(name="rbp", bufs=4) as rbp,
            tc.tile_pool(name="normp", bufs=1) as normp,
            tc.tile_pool(name="outsb", bufs=12) as outsb,
            tc.tile_pool(name="ps_proj", bufs=2, space="PSUM") as ps_proj,
            tc.tile_pool(name="ps_o", bufs=2, space="PSUM") as ps_o,
            tc.tile_pool(name="ps_pair", bufs=2, space="PSUM") as ps_pair,
        ):
            # ---- loads. x is packed seq-block-major ([128, 6e x 512] per
            # 512-sequence block) so the first K/Q chunk only needs wk + one
            # 0.75MB block; wk + block 0/1 load first, k/q chunks of head 0
            # then pipeline behind the remaining block DMAs. ----
            XB = NE * 512  # 3072 cols per seq block
            wk0_sb = constp.tile([128, NE * HDP], bf, name="wk0_sb")
            nc.sync.dma_start(out=wk0_sb, in_=wkp0)
            xtb = [bigp.tile([128, XB], bf, name=f"xtb{n}") for n in range(4)]
            # the first x half rides the Pool/SWDGE queue: its descriptor
            # path is independent of the HWDGE slot the weight loads use,
            # so it overlaps wk0 end-to-end
            nc.gpsimd.dma_start(out=xtb[0][:, 0:XB // 2],
                                in_=xtp[:, 0:XB // 2])
            nc.sync.dma_start(out=xtb[0][:, XB // 2:XB],
                              in_=xtp[:, XB // 2:XB])
            # biases next: tiny, but they gate the K/Q psum evictions
            bq_sb = constp.tile([128, HPC], f32, name="bq_sb")
            nc.sync.dma_start(out=bq_sb, in_=bqp)
            bk_sb = constp.tile([128, HPC], f32, name="bk_sb")
            nc.sync.dma_start(out=bk_sb, in_=bkp)
            wq0_sb = constp.tile([128, NE * HDP], bf, name="wq0_sb")
            nc.sync.dma_start(out=wq0_sb, in_=wqp0)
            wv_sb = constp.tile([128, NE * HPC * HD], bf, name="wv_sb")
            nc.sync.dma_start(out=wv_sb, in_=wvp)
            nc.sync.dma_start(out=xtb[1], in_=xtp[:, XB:2 * XB])
            wkr_sb = constp.tile([128, NE * 3 * HDP], bf, name="wkr_sb")
            nc.sync.dma_start(out=wkr_sb, in_=wkpr)
            wqr_sb = constp.tile([128, NE * 3 * HDP], bf, name="wqr_sb")
            nc.sync.dma_start(out=wqr_sb, in_=wqpr)
            nc.sync.dma_start(out=xtb[2], in_=xtp[:, 2 * XB:3 * XB])
            nc.sync.dma_start(out=xtb[3], in_=xtp[:, 3 * XB:4 * XB])
            wo_sb = constp.tile([128, 3 * EMB], bf, name="wo_sb")
            nc.sync.dma_start(out=wo_sb, in_=wop)
            # identity for the PE repack transposes (first needed in head
            # 3's phase, so it loads last)
            ident_sb = constp.tile([128, 128], bf, name="ident_sb")
            nc.sync.dma_start(out=ident_sb, in_=identp)

            def wk_eh(e, h):
                if h == 0:
                    return wk0_sb[:, e * HDP:(e + 1) * HDP]
                return wkr_sb[:, (e * 3 + h - 1) * HDP:(e * 3 + h) * HDP]

            def wq_eh(e, h):
                if h == 0:
                    return wq0_sb[:, e * HDP:(e + 1) * HDP]
                return wqr_sb[:, (e * 3 + h - 1) * HDP:(e * 3 + h) * HDP]

            def wv_e(e):
                return wv_sb[:, e * HPC * HD:(e + 1) * HPC * HD]

            def wo_t(t_):
                return wo_sb[:, t_ * EMB:(t_ + 1) * EMB]

            # ---- persistent intermediates ----
            # vaug: per key tile, 4 heads x (96 v-cols + a ones col).  The
            # ones col makes row^T @ vaug yield the softmax denominator in
            # the same accumulator (col 96 of each head group).  No pad
            # cols: vaug is the MOVING operand of the flipped O matmul, so
            # narrower means cheaper (97 vs 128 cycles).
            HDV = HD + 1
            vaug = []
            for kt in range(NKT):
                t = bigp.tile([128, HPC * HDV], bf, name=f"vaug{kt}")
                ones_cols = t.rearrange("p (h c) -> p h c",
                                        h=HPC)[:, :, HD:HD + 1]
                nc.gpsimd.memset(ones_cols, 1.0)
                vaug.append(t)
            qT = [bigp.tile([128, SEQ], bf, name=f"qT{h}") for h in range(HPC)]
            kT = [bigp.tile([128, SEQ], bf, name=f"kT{h}") for h in range(HPC)]
            # packed attention output, [384 rows = 3 tiles x 128, seq],
            # written by the h3-phase repack transposes
            attnT = [bigp.tile([128, SEQ], bf, name=f"attnT{t_}")
                     for t_ in range(3)]
            # normalized O in [q, head-dims] layout, per (q-chunk, q-tile):
            # written per head as its window completes, transposed into
            # attnT once all four heads are in
            normed = [[normp.tile([128, HPC * HD], bf, name=f"nm{qc}_{qt}")
                       for qt in range(4)] for qc in range(NQC)]

            f32_ = f32

            # ---- projection emit helpers ----
            def emit_v_chunk(kt):
                psv = ps_proj.tile([128, 512], f32_, tag="ps",
                                   name=f"psv{kt}")
                blk, off = divmod(kt, 4)
                for e in range(NE):
                    nc.tensor.matmul(psv[:, 0:HPC * HD],
                                     lhsT=xtb[blk][:, e * 512 + off * 128:
                                                   e * 512 + off * 128 + 128],
                                     rhs=wv_e(e),
                                     start=(e == 0), stop=(e == NE - 1))
                for hh in range(HPC):
                    nc.vector.tensor_copy(
                        vaug[kt][:, hh * HDV:hh * HDV + HD],
                        psv[:, hh * HD:(hh + 1) * HD])

            kq_ps = {}

            def emit_kq_part(h, n, which, part):
                """One half of a K/Q chunk (3 of 6 e-matmuls).  Thunking
                chunks at this granularity keeps the per-pair PE load above
                the ACT exp cadence, so the S stream never outruns ACT into
                the 2-slot pss ring."""
                key = (h, n, which)
                w_eh, dst, b_sb = ((wk_eh, kT, bk_sb) if which == "k"
                                   else (wq_eh, qT, bq_sb))
                if part == 0:
                    kq_ps[key] = ps_proj.tile([128, 512], f32_, tag="ps",
                                              name=f"ps{which}{h}_{n}")
                ps = kq_ps[key]
                for e in (3 * part, 3 * part + 1, 3 * part + 2):
                    nc.tensor.matmul(ps,
                                     lhsT=w_eh(e, h),
                                     rhs=xtb[n][:, e * 512:(e + 1) * 512],
                                     start=(e == 0), stop=(e == NE - 1))
                if part == 1:
                    nsl = slice(n * 512, (n + 1) * 512)
                    nc.vector.tensor_scalar_add(dst[h][:, nsl],
                                                kq_ps.pop(key),
                                                b_sb[:, h:h + 1])

            def emit_kq_chunk(h, n, which):
                for part in range(2):
                    emit_kq_part(h, n, which, part)

            def kq_chunks(h):
                for n in range(4):
                    yield ("k", h, n)
                    yield ("q", h, n)

            # ---- output projection chunk (one 128-row q tile) ----
            # Split across two 1-bank psums so it can borrow ps_proj slots;
            # PSUM->SBUF copies go on DVE (ACT is busy with exp here).
            # Tail out-proj chunks, two-phase: head 3's rows live only in
            # packed attnT tile 2, so the t0/t1 matmuls are independent of
            # the final normalization and run DURING its chain; only the t2
            # matmul (accumulation close) waits for the per-qm norm slice.
            # Psums borrow the attention rings (free by then).
            tail_ps = {}

            def emit_out_partial(qm, mode="tail"):
                qsl = slice(qm * 128, (qm + 1) * 128)
                if mode == "win":
                    psA = ps_proj.tile([128, 512], f32_, tag="ps",
                                       name=f"poA{qm}")
                    psB = ps_proj.tile([128, 512], f32_, tag="ps",
                                       name=f"poB{qm}")
                elif mode == "edge":
                    # last-window partial: psA on the ps ring, psB on the
                    # pso ring's spare slot (the previous psof freed once
                    # its normalization drained) so only ONE ps slot is
                    # held across the window/tail boundary
                    psA = ps_proj.tile([128, 512], f32_, tag="ps",
                                       name=f"poA{qm}")
                    psB = ps_o.tile([128, 512], f32_, tag="pso",
                                    name=f"poB{qm}")
                else:
                    # tail: both halves on the pss ring (free then)
                    psA = ps_pair.tile([128, 512], f32_, tag="pss",
                                       name=f"poA{qm}")
                    psB = ps_pair.tile([128, 512], f32_, tag="pss",
                                       name=f"poB{qm}")
                tail_ps[qm] = (psA, psB)
                for t in range(2):
                    nc.tensor.matmul(psA,
                                     lhsT=attnT[t][:, qsl],
                                     rhs=wo_t(t)[:, 0:512],
                                     start=(t == 0), stop=False)
                    nc.tensor.matmul(psB[:, 0:256],
                                     lhsT=attnT[t][:, qsl],
                                     rhs=wo_t(t)[:, 512:768],
                                     start=(t == 0), stop=False)

            def emit_out_finish(qm, windowed=False):
                qsl = slice(qm * 128, (qm + 1) * 128)
                psA, psB = tail_ps.pop(qm)
                nc.tensor.matmul(psA, lhsT=attnT[2][:, qsl],
                                 rhs=wo_t(2)[:, 0:512],
                                 start=False, stop=True)
                nc.tensor.matmul(psB[:, 0:256], lhsT=attnT[2][:, qsl],
                                 rhs=wo_t(2)[:, 512:768],
                                 start=False, stop=True)
                out_sb = outsb.tile([128, EMB], f32_, tag="osb",
                                    name=f"osb{qm}")
                # evictions on DVE in windows (ACT is exp-saturated there;
                # GPSIMD cannot touch PSUM on real hw).  In the tail, psB
                # rides ACT end-to-end (evict + store) so the last stores
                # split across the SP and ACT queues instead of
                # serializing on SP's 650 ns issue cost.
                nc.vector.tensor_copy(out_sb[:, 0:512], psA)
                if windowed:
                    nc.sync.dma_start(
                        out=outp[qm * 128:(qm + 1) * 128, 0:512],
                        in_=out_sb[:, 0:512])
                else:
                    eng = nc.gpsimd if qm % 2 == 0 else nc.sync
                    eng.dma_start(
                        out=outp[qm * 128:(qm + 1) * 128, 0:512],
                        in_=out_sb[:, 0:512])
                if windowed:
                    nc.vector.tensor_copy(out_sb[:, 512:768], psB[:, 0:256])
                    nc.sync.dma_start(
                        out=outp[qm * 128:(qm + 1) * 128, 512:768],
                        in_=out_sb[:, 512:768])
                else:
                    # tail: psB evict on DVE; store rotated across the
                    # ACT and Pool/SWDGE queues so the four final stores
                    # don't serialize on the single HWDGE slot
                    nc.vector.tensor_copy(out_sb[:, 512:768], psB[:, 0:256])
                    eng = nc.scalar if qm % 2 == 0 else nc.gpsimd
                    eng.dma_start(
                        out=outp[qm * 128:(qm + 1) * 128, 512:768],
                        in_=out_sb[:, 512:768])

            # ---- attention emit (with interleaved PE filler work) ----
            # repack: transpose normed[qc][qt] ([128 q, 384 packed head
            # dims]) into attnT via 128x128 PE transposes + DVE evictions.
            # Chunks t0/t1 cover only heads 0-2, so they repack as soon as
            # head 2's normalization lands - during head 3's own windows -
            # and the out-proj partials (which read t0/t1) can then run
            # inside the last window.  Only the t2 chunk (heads 2+3) waits
            # for head 3's normalization.
            def emit_repack01(qc, qt):
                tps = ps_proj.tile([128, 2 * 128], bf, tag="ps",
                                   name=f"tp01_{qc}_{qt}")
                for t_ in range(2):
                    nc.tensor.transpose(
                        tps[:, t_ * 128:(t_ + 1) * 128],
                        normed[qc][qt][:, t_ * 128:(t_ + 1) * 128],
                        ident_sb)
                csl = slice(qc * QC + qt * 128, qc * QC + (qt + 1) * 128)
                for t_ in range(2):
                    nc.vector.tensor_copy(attnT[t_][:, csl],
                                          tps[:, t_ * 128:(t_ + 1) * 128])

            def emit_repack2(qc, qt, tail=False):
                tps = ps_proj.tile([128, 128], bf, tag="ps",
                                   name=f"tp2_{qc}_{qt}")
                nc.tensor.transpose(tps, normed[qc][qt][:, 256:384],
                                    ident_sb)
                csl = slice(qc * QC + qt * 128, qc * QC + (qt + 1) * 128)
                if tail:
                    # keep the tail's DVE queue clear for the out evictions
                    nc.scalar.activation(attnT[2][:, csl], tps,
                                         mybir.ActivationFunctionType.Copy)
                else:
                    nc.vector.tensor_copy(attnT[2][:, csl], tps)

            # cross-window O backlog: each pair's O-block (and, for a
            # window's last pair, its normalization) is queued and drained
            # `lag` pairs later - ACROSS window boundaries for heads 0-2,
            # so a window's trailing O's overlap the next window's S
            # stream.  Head-3 windows drain fully at their end because the
            # following window's repack thunks need the normalization.
            obl = []

            def emit_attention(h, thunks_for_qc):
                """thunks_for_qc(qc) -> list of emit callables injected into
                the PE stream spread across this q-chunk's pairs."""
                for qc in range(NQC):
                    thunks, pos, post = thunks_for_qc(qc)
                    inject_at = {}
                    if pos is not None:
                        for i, t in enumerate(thunks):
                            inject_at.setdefault(
                                pos[i] if i < len(pos) else NPAIR - 1,
                                []).append(t)
                    else:
                        for i, t in enumerate(thunks):
                            inject_at.setdefault(
                                min(2 * i, NPAIR - 1), []).append(t)
                    qsl = slice(qc * QC, (qc + 1) * QC)
                    idx = h * NQC + qc
                    # flipped-O accumulator: [128 q, 4 qt x (96 d + sums)]
                    # packed per q-tile into one psum bank.  The bank holds
                    # FOUR interleaved accumulation chains; hardware supports
                    # only one OPEN start/stop group per bank at a time
                    # (later start=True resets the others), so the tile is
                    # zeroed up front and every matmul accumulates with
                    # start=False.
                    psof = ps_o.tile([128, 4 * HDV], f32_, tag="pso",
                                     name=f"psof{idx}")
                    nc.vector.memset(psof, 0.0)
                    eps = []

                    def emit_ss(p):
                        pss = ps_pair.tile([128, 1024], f32_, tag="pss",
                                           name=f"pss{idx}_{p}")
                        ep = expp.tile([128, 1024], bf, tag="exp",
                                       name=f"exp{idx}_{p}")
                        # exp split: Pool takes the FIRST cols (waits only
                        # on the j=0 matmul, so its q7-launch latency hides
                        # behind the j=1 matmul), ACT takes the rest.  Both
                        # cadences stay under the pair's PE work so PE is
                        # the limiter.  Head 3 (qc>0) keeps exp fully on ACT
                        # - its windows carry out-proj thunks (PE/pair well
                        # above 1038 ns) and Pool absorbs psum evictions.
                        # both S matmuls BEFORE either exp: an exp emitted
                        # between them reads this pss tile and Tile's
                        # conservative subtile tracking then serializes the
                        # second matmul behind it (false intra-tile WAR)
                        nc.tensor.matmul(
                            pss[:, 0:512],
                            lhsT=kT[h][:, (2 * p) * 128:(2 * p + 1) * 128],
                            rhs=qT[h][:, qsl],
                            start=True, stop=True)
                        nc.tensor.matmul(
                            pss[:, 512:1024],
                            lhsT=kT[h][:, (2 * p + 1) * 128:
                                       (2 * p + 2) * 128],
                            rhs=qT[h][:, qsl],
                            start=True, stop=True)
                        nc.scalar.activation(
                            ep, pss,
                            mybir.ActivationFunctionType.Exp,
                            scale=SCALING)
                        eps.append(ep)

                    def emit_o(p, ep, psof=psof, h=h):
                        # flipped: the exp tile is the (free) stationary
                        # load, the 97-col vaug slab streams -> 40 ns per
                        # matmul instead of 213, output lands [q, d]
                        for j in range(2):
                            kt = 2 * p + j
                            for qt in range(4):
                                nc.tensor.matmul(
                                    psof[:, qt * HDV:(qt + 1) * HDV],
                                    lhsT=ep[:, j * 512 + qt * 128:
                                            j * 512 + (qt + 1) * 128],
                                    rhs=vaug[kt][:, h * HDV:(h + 1) * HDV],
                                    start=False, stop=(kt == NKT - 1))

                    def emit_norm(psof=psof, h=h, qc=qc, idx=idx):
                        # sums live at col 96 of each q-tile group; the
                        # reciprocal is a per-partition scalar.  One
                        # contiguous single-column reciprocal per q-tile
                        # (the custom-DVE op mis-lowers strided APs).
                        rbq = rbp.tile([128, 4], f32_, tag="rbq",
                                       name=f"rbq{idx}")
                        for qt in range(4):
                            nc.vector.reciprocal_approx_fast(
                                out=rbq[:, qt:qt + 1],
                                in_=psof[:, qt * HDV + HD:qt * HDV + HD + 1])
                        for qt in range(4):
                            nc.vector.tensor_scalar_mul(
                                normed[qc][qt][:, h * HD:(h + 1) * HD],
                                psof[:, qt * HDV:qt * HDV + HD],
                                rbq[:, qt:qt + 1])

                    def make_o(p, last, ep):
                        def f():
                            emit_o(p, ep)
                            if last:
                                emit_norm()
                        return f

                    lag = 3 if (h == HPC - 1 and qc == NQC - 1) else 4
                    for p in range(NPAIR):
                        emit_ss(p)
                        for t in inject_at.get(p, ()):
                            t()
                        obl.append(make_o(p, p == NPAIR - 1, eps[-1]))
                        while len(obl) > lag:
                            obl.pop(0)()
                    # drain the O backlog woven with any held-back thunks:
                    # the last pairs' exps finish ~1 us after the S stream,
                    # so the interleaved thunks keep the PE busy across
                    # that latency and the following norm chain
                    post = list(post)
                    while obl:
                        obl.pop(0)()
                        if len(obl) <= 2 and post:
                            post.pop(0)()
                    while post:
                        post.pop(0)()

            # ---- emission schedule ----
            # consume each x seq-block as its DMA lands: head 0's k/q chunks
            # for block n, then the V chunks of block n
            for n in range(4):
                emit_kq_chunk(0, n, "k")
                emit_kq_chunk(0, n, "q")
                for kt in range(4 * n, 4 * n + 4):
                    emit_v_chunk(kt)

            def kq_thunks(hnext):
                def f(qc):
                    # 2 chunks per q-chunk, split into 3-matmul halves (4
                    # thunks of ~640 ns) so the PE filler spreads across the
                    # window instead of lumping at two pairs
                    items = list(kq_chunks(hnext))[2 * qc:2 * qc + 2]
                    out = []
                    for it in items:
                        for part in range(2):
                            out.append(
                                lambda it=it, part=part: emit_kq_part(
                                    it[1], it[2], it[0], part))
                    return out, None, []
                return f

            def rp01_thunks(qc):
                return [lambda qt=qt: emit_repack01(qc, qt)
                        for qt in range(4)]

            def out_thunks(qc):
                # head-3 window qc: close out q-chunk qc-1 (t2 repack + P/F
                # two-phase out-proj) and pre-repack t0/t1 of q-chunk qc+1
                # (they only need heads 0-2, normalized long ago).  The
                # last window also starts P(q0+0) with its psB on the pso
                # spare slot so the tail begins with a partial in flight.
                if qc == 0:
                    return (rp01_thunks(0) + rp01_thunks(1)
                            + rp01_thunks(2),
                            [0, 1, 1, 2, 3, 3, 4, 5, 5, 6, 7, 7], [])
                qp = qc - 1
                out = []
                for qt4 in range(0, 4, 2):
                    out.append(lambda qt=qt4: emit_repack2(qp, qt))
                    out.append(lambda qt=qt4 + 1: emit_repack2(qp, qt))
                    for qm in (4 * qp + qt4, 4 * qp + qt4 + 1):
                        out.append(
                            lambda qm=qm: emit_out_partial(qm, "win"))
                        out.append(lambda qm=qm: emit_out_finish(qm, True))
                pos = [0, 0, 1, 1, 2, 2, 3, 3, 4, 4, 5, 5]
                if qc == NQC - 2:
                    out += rp01_thunks(qc + 1)
                    pos += [6, 6, 7, 7]
                elif qc == NQC - 1:
                    out.append(lambda: emit_out_partial(4 * qc, "edge"))
                    pos += [7]
                return out, pos, []

            for h in range(HPC - 1):
                emit_attention(h, kq_thunks(h + 1))
            emit_attention(HPC - 1, out_thunks)
            # tail: only the t2 repacks and the finish halves remain; each
            # F(qm) frees the ps/pso slots its successor partial needs
            q0 = 4 * (NQC - 1)
            emit_repack2(NQC - 1, 0, True)
            emit_out_finish(q0)
            emit_out_partial(q0 + 1)
            emit_repack2(NQC - 1, 1, True)
            emit_out_finish(q0 + 1)
            emit_out_partial(q0 + 2)
            emit_repack2(NQC - 1, 2, True)
            emit_out_finish(q0 + 2)
            emit_out_partial(q0 + 3)
            emit_repack2(NQC - 1, 3, True)
            emit_out_finish(q0 + 3)

            if os.environ.get("KDEBUG"):
                dbg_attn = nc.dram_tensor(
                    "dbg_attn", [3 * 128, SEQ], bf,
                    kind="ExternalOutput").ap()
                for t_ in range(3):
                    nc.sync.dma_start(
                        out=dbg_attn[t_ * 128:(t_ + 1) * 128, :],
                        in_=attnT[t_])
                dbg_nm = nc.dram_tensor(
                    "dbg_nm", [NQC * 4 * 128, HPC * HD], bf,
                    kind="ExternalOutput").ap()
                for qc in range(NQC):
                    for qt in range(4):
                        r0 = (qc * 4 + qt) * 128
                        nc.sync.dma_start(out=dbg_nm[r0:r0 + 128, :],
                                          in_=normed[qc][qt])

    nc.compile()
    return nc


def _get_nc():
    if "nc" not in _NC_CACHE:
        _NC_CACHE["nc"] = _build_nc()
    return _NC_CACHE["nc"]


def _etile_pack(wT):
    """[768, n] (e on rows) -> [128, 6*n] bf16: e-tiles as column blocks so
    the whole operand loads as ONE [128, n] DMA."""
    n = wT.shape[1]
    a = wT.reshape(NE, 128, n).transpose(1, 0, 2)
    return np.ascontiguousarray(a.reshape(128, NE * n)).astype(BF16)


def _x_block_pack(x_b):
    """[2048, 768] x -> [128, 4 * 6 * 512] bf16, seq-block-major: block n
    holds e-tiles of sequence rows n*512..(n+1)*512 as column slabs."""
    a = x_b.reshape(4, 512, NE, 128)          # n, s, e, p
    a = a.transpose(3, 0, 2, 1)               # p, n, e, s
    return np.ascontiguousarray(a.reshape(128, 4 * NE * 512)).astype(BF16)


def _pad_headsT(w_rows):
    """[384, 768] head rows -> zero-pad head dim 96->128 -> transpose -> [768, 512]."""
    p = np.zeros((HPC * HDP, EMB), np.float32)
    p.reshape(HPC, HDP, EMB)[:, :HD] = w_rows.reshape(HPC, HD, EMB)
    return np.ascontiguousarray(p.T)


def _pad_bias(b_rows):
    """[384] head bias -> [128, HPC] padded/transposed for per-partition add."""
    p = np.zeros((HPC, HDP), np.float32)
    p[:, :HD] = b_rows.reshape(HPC, HD)
    return np.ascontiguousarray(p.T)


def kernel(x, Wq, bq, Wk, bk, Wv, bv, Wo, bo):
    x = np.asarray(x, np.float32)
    Wq, bq = np.asarray(Wq, np.float32), np.asarray(bq, np.float32)
    Wk, bk = np.asarray(Wk, np.float32), np.asarray(bk, np.float32)
    Wv, bv = np.asarray(Wv, np.float32), np.asarray(bv, np.float32)
    Wo, bo = np.asarray(Wo, np.float32), np.asarray(bo, np.float32)

    nc = _get_nc()

    in_maps = []
    for c in range(NCORES):
        b, hg = divmod(c, 2)
        hs = slice(hg * HPC * HD, (hg + 1) * HPC * HD)
        woT = Wo[:, hs].T  # [384, 768]
        wo_pack = np.ascontiguousarray(
            woT.reshape(3, 128, EMB).transpose(1, 0, 2).reshape(128, 3 * EMB))
        wq_et = _etile_pack(_pad_headsT(Wq[hs])).reshape(128, NE, HPC, HDP)
        wk_et = _etile_pack(_pad_headsT(Wk[hs])).reshape(128, NE, HPC, HDP)
        in_maps.append({
            "xtp": _x_block_pack(x[b]),
            "wqp0": np.ascontiguousarray(
                wq_et[:, :, 0].reshape(128, NE * HDP)),
            "wqpr": np.ascontiguousarray(
                wq_et[:, :, 1:].reshape(128, NE * 3 * HDP)),
            "wkp0": np.ascontiguousarray(
                wk_et[:, :, 0].reshape(128, NE * HDP)),
            "wkpr": np.ascontiguousarray(
                wk_et[:, :, 1:].reshape(128, NE * 3 * HDP)),
            "wvp": _etile_pack(np.ascontiguousarray(Wv[hs].T)),
            "wop": wo_pack.astype(BF16),
            "bqp": _pad_bias(bq[hs]),
            "bkp": _pad_bias(bk[hs]),
            "identp": np.ascontiguousarray(np.eye(128, dtype=np.float32))
            .astype(BF16),
        })

    global LAST_RESULT
    trace = bool(int(os.environ.get("KERNEL_TRACE", "0")))
    tmpdir = os.environ.get("KERNEL_TRACE_DIR") or None
    res = run_bass_kernel_spmd(nc, in_maps, list(range(NCORES)), trace=trace,
                               tmpdir=tmpdir)
    LAST_RESULT = res

    out = np.empty((B, SEQ, EMB), np.float32)
    for b in range(B):
        out[b] = res.results[2 * b]["outp"] + res.results[2 * b + 1]["outp"]
    # bv enters each head's output additively (sum of softmax weights is 1),
    # and bo is a plain add: both fold into one constant vector.
    out += Wo @ bv + bo
    return out



# revision 141
# speedup vs baseline: 1.0052x; 1.0039x over previous
"""Multi-head attention (B=4, S=2048, E=768, H=8, D=96) on 8 Trainium2 cores.

Sharding: core c -> (batch b = c//2, head-group hg = c%2 of 4 heads).
Each core computes Q/K/V projections for its 4 heads over the full sequence
of its batch, full attention for those heads, and a partial output
projection (row-split Wo).  The two cores of a batch produce partial
outputs that are summed on the host during unsharding (tensor-parallel
reduce).

On-chip layout notes:
  - All matmul operands are bf16 (1 cycle/row on PE; fp32 would be 4x; fp8
    DoubleRow would halve PE time but its ~3% per-element noise lands ~1:1
    in the output and busts the 2e-2 tolerance).
  - Scores are computed transposed, S^T[k, q] = K^T.T @ Q^T (the K/Q
    projections pad head dim 96->128 on the contraction partitions, which
    costs nothing).  exp(S) runs on ACT straight out of PSUM over a
    [128, 1024] pair of key tiles with the 1/sqrt(d) scale folded in.
    ACT's 1038 ns per pair is the cadence ceiling for bare windows.
  - The O = V^T exp(S) matmul is FLIPPED: the exp tile is the stationary
    operand (Ldweights is free) and a 97-column vaug slab (96 V dims + a
    ones column) streams, so each matmul costs 40 ns instead of 213 and
    the per-pair PE work drops from 852 to 749 ns.  The ones column makes
    the softmax denominator fall out at column 96 of each q-tile group of
    the [q, 4 x 97] accumulator, so normalization is a per-partition
    reciprocal + tensor_scalar - no broadcast machinery at all.
  - A PSUM bank supports only ONE open accumulation group at a time, so
    the flipped accumulator (4 interleaved chains in one bank) is zeroed
    up front and every matmul accumulates with start=False.
  - The normalized [q, d] output is transposed back to the [d, q] layout
    the output projection needs via 128x128 PE transposes (53 ns each)
    against a host-provided identity.  attnT chunks t0/t1 only contain
    heads 0-2, so they repack during head 3's own windows (the ACT-bound
    first window absorbs three q-chunks' worth as PE filler); only the t2
    chunk waits for head 3's normalization, which keeps the tail short.
  - O matmuls consume exps with a 4-pair lag so the ACT exp latency
    (~1.35 us from S matmul to usable exp) never stalls the PE.  Both S
    matmuls of a pair are emitted BEFORE the exp that reads them: Tile's
    conservative subtile tracking would otherwise serialize the second
    matmul behind the exp (false intra-tile WAR).
  - Inputs are host-packed so each operand is ONE [128, n] DMA.  x is
    packed seq-block-major; the first x half rides the Pool/SWDGE queue
    (independent of the HWDGE slot) so it overlaps the wk0 load; head 0's
    K/Q weight columns load before the other heads'.
  - PE stream order: head 0's K/Q projections and V chunks per x block,
    then 16 attention windows (4 heads x 4 q-chunks).  Heads 0-2 carry
    the next head's K/Q chunks as thunked PE filler; head 3's windows
    carry the previous q-chunk's t2 repack + two-phase output projection
    (P = t0/t1 partials, F = t2 close + DVE evictions + stores split
    across the SP/ACT/SWDGE queues).  PSUM rings: pss pairs (2x2 banks),
    flipped-O accumulators (2x1), projection/repack ring (2x1); every
    allocation is placed one eviction behind its ring slot's previous
    user.
"""

import os
import sys

sys.path.insert(0, "/opt/trn_rl_repo")

import numpy as np
import ml_dtypes

import concourse.bacc as bacc
import concourse.tile as tile
from concourse import mybir
from concourse.bass_utils import run_bass_kernel_spmd

BF16 = ml_dtypes.bfloat16

EMB = 768
HEADS = 8
HD = 96          # true head dim
HDP = 128        # padded head dim
SEQ = 2048
B = 4
NCORES = 8
HPC = 4          # heads per core
SCALING = HD ** -0.5
QC = 512         # query chunk per attention inner loop
NQC = SEQ // QC
NKT = SEQ // 128  # 16 key tiles
NPAIR = NKT // 2
NE = EMB // 128   # 6 e_in tiles

_NC_CACHE = {}
LAST_RESULT = None  # BassKernelResults of the most recent run (for test.py)


def _build_nc():
    f32 = mybir.dt.float32
    bf = mybir.dt.bfloat16

    nc = bacc.Bacc(trn_type="TRN2", target_bir_lowering=False, debug=False,
                   num_devices=NCORES)

    # All operands host-packed into [128, n] so each loads as ONE DMA.
    xtp = nc.dram_tensor("xtp", [128, NE * SEQ], bf, kind="ExternalInput").ap()
    # K/Q weights split: head 0's columns load first (0.19MB) so head 0's
    # projections - which gate the whole pipeline - start ~5 us earlier
    wqp0 = nc.dram_tensor("wqp0", [128, NE * HDP], bf,
                          kind="ExternalInput").ap()
    wqpr = nc.dram_tensor("wqpr", [128, NE * 3 * HDP], bf,
                          kind="ExternalInput").ap()
    wkp0 = nc.dram_tensor("wkp0", [128, NE * HDP], bf,
                          kind="ExternalInput").ap()
    wkpr = nc.dram_tensor("wkpr", [128, NE * 3 * HDP], bf,
                          kind="ExternalInput").ap()
    wvp = nc.dram_tensor("wvp", [128, NE * HPC * HD], bf,
                         kind="ExternalInput").ap()
    wop = nc.dram_tensor("wop", [128, 3 * EMB], bf, kind="ExternalInput").ap()
    bqp = nc.dram_tensor("bqp", [128, HPC], f32, kind="ExternalInput").ap()
    bkp = nc.dram_tensor("bkp", [128, HPC], f32, kind="ExternalInput").ap()
    identp = nc.dram_tensor("identp", [128, 128], bf,
                            kind="ExternalInput").ap()
    outp = nc.dram_tensor("outp", [SEQ, EMB], f32, kind="ExternalOutput").ap()

    with tile.TileContext(nc) as tc:
        with (
            tc.tile_pool(name="const", bufs=1) as constp,
            tc.tile_pool(name="big", bufs=1) as bigp,
            tc.tile_pool(name="expp", bufs=6) as expp,
            tc.tile_pool(name="rbp", bufs=4) as rbp,
            tc.tile_pool(name="normp", bufs=1) as normp,
            tc.tile_pool(name="outsb", bufs=12) as outsb,
            tc.tile_pool(name="ps_proj", bufs=2, space="PSUM") as ps_proj,
            tc.tile_pool(name="ps_o", bufs=2, space="PSUM") as ps_o,
            tc.tile_pool(name="ps_pair", bufs=2, space="PSUM") as ps_pair,
        ):
            # ---- loads. x is packed seq-block-major ([128, 6e x 512] per
            # 512-sequence block) so the first K/Q chunk only needs wk + one
            # 0.75MB block; wk + block 0/1 load first, k/q chunks of head 0
            # then pipeline behind the remaining block DMAs. ----
            XB = NE * 512  # 3072 cols per seq block
            wk0_sb = constp.tile([128, NE * HDP], bf, name="wk0_sb")
            nc.sync.dma_start(out=wk0_sb, in_=wkp0)
            xtb = [bigp.tile([128, XB], bf, name=f"xtb{n}") for n in range(4)]
            # the first x half rides the Pool/SWDGE queue: its descriptor
            # path is independent of the HWDGE slot the weight loads use,
            # so it overlaps wk0 end-to-end
            nc.gpsimd.dma_start(out=xtb[0][:, 0:XB // 2],
                                in_=xtp[:, 0:XB // 2])
            nc.sync.dma_start(out=xtb[0][:, XB // 2:XB],
                              in_=xtp[:, XB // 2:XB])
            # biases next: tiny, but they gate the K/Q psum evictions
            bq_sb = constp.tile([128, HPC], f32, name="bq_sb")
            nc.sync.dma_start(out=bq_sb, in_=bqp)
            bk_sb = constp.tile([128, HPC], f32, name="bk_sb")
            nc.sync.dma_start(out=bk_sb, in_=bkp)
            wq0_sb = constp.tile([128, NE * HDP], bf, name="wq0_sb")
            nc.sync.dma_start(out=wq0_sb, in_=wqp0)
            wv_sb = constp.tile([128, NE * HPC * HD], bf, name="wv_sb")
            nc.sync.dma_start(out=wv_sb, in_=wvp)
            nc.sync.dma_start(out=xtb[1], in_=xtp[:, XB:2 * XB])
            wkr_sb = constp.tile([128, NE * 3 * HDP], bf, name="wkr_sb")
            nc.sync.dma_start(out=wkr_sb, in_=wkpr)
            wqr_sb = constp.tile([128, NE * 3 * HDP], bf, name="wqr_sb")
            nc.sync.dma_start(out=wqr_sb, in_=wqpr)
            nc.sync.dma_start(out=xtb[2], in_=xtp[:, 2 * XB:3 * XB])
            nc.sync.dma_start(out=xtb[3], in_=xtp[:, 3 * XB:4 * XB])
            wo_sb = constp.tile([128, 3 * EMB], bf, name="wo_sb")
            nc.sync.dma_start(out=wo_sb, in_=wop)
            # identity for the PE repack transposes (first needed in head
            # 3's phase, so it loads last)
            ident_sb = constp.tile([128, 128], bf, name="ident_sb")
            nc.sync.dma_start(out=ident_sb, in_=identp)

            def wk_eh(e, h):
                if h == 0:
                    return wk0_sb[:, e * HDP:(e + 1) * HDP]
                return wkr_sb[:, (e * 3 + h - 1) * HDP:(e * 3 + h) * HDP]

            def wq_eh(e, h):
                if h == 0:
                    return wq0_sb[:, e * HDP:(e + 1) * HDP]
                return wqr_sb[:, (e * 3 + h - 1) * HDP:(e * 3 + h) * HDP]

            def wv_e(e):
                return wv_sb[:, e * HPC * HD:(e + 1) * HPC * HD]

            def wo_t(t_):
                return wo_sb[:, t_ * EMB:(t_ + 1) * EMB]

            # ---- persistent intermediates ----
            # vaug: per key tile, 4 heads x (96 v-cols + a ones col).  The
            # ones col makes row^T @ vaug yield the softmax denominator in
            # the same accumulator (col 96 of each head group).  No pad
            # cols: vaug is the MOVING operand of the flipped O matmul, so
            # narrower means cheaper (97 vs 128 cycles).
            HDV = HD + 1
            vaug = []
            for kt in range(NKT):
                t = bigp.tile([128, HPC * HDV], bf, name=f"vaug{kt}")
                ones_cols = t.rearrange("p (h c) -> p h c",
                                        h=HPC)[:, :, HD:HD + 1]
                nc.gpsimd.memset(ones_cols, 1.0)
                vaug.append(t)
            qT = [bigp.tile([128, SEQ], bf, name=f"qT{h}") for h in range(HPC)]
            kT = [bigp.tile([128, SEQ], bf, name=f"kT{h}") for h in range(HPC)]
            # packed attention output, [384 rows = 3 tiles x 128, seq],
            # written by the h3-phase repack transposes
            attnT = [bigp.tile([128, SEQ], bf, name=f"attnT{t_}")
                     for t_ in range(3)]
            # normalized O in [q, head-dims] layout, per (q-chunk, q-tile):
            # written per head as its window completes, transposed into
            # attnT once all four heads are in
            normed = [[normp.tile([128, HPC * HD], bf, name=f"nm{qc}_{qt}")
                       for qt in range(4)] for qc in range(NQC)]

            f32_ = f32

            # ---- projection emit helpers ----
            def emit_v_chunk(kt):
                psv = ps_proj.tile([128, 512], f32_, tag="ps",
                                   name=f"psv{kt}")
                blk, off = divmod(kt, 4)
                for e in range(NE):
                    nc.tensor.matmul(psv[:, 0:HPC * HD],
                                     lhsT=xtb[blk][:, e * 512 + off * 128:
                                                   e * 512 + off * 128 + 128],
                                     rhs=wv_e(e),
                                     start=(e == 0), stop=(e == NE - 1))
                # one strided copy for all four heads: 525 ns instead of
                # 4 x 225, and the ps-ring slot frees a full copy earlier
                nc.vector.tensor_copy(
                    vaug[kt].rearrange("p (h c) -> p h c",
                                       h=HPC)[:, :, 0:HD],
                    psv[:, 0:HPC * HD].rearrange("p (h c) -> p h c",
                                                 h=HPC))

            kq_ps = {}

            def emit_kq_part(h, n, which, part):
                """One half of a K/Q chunk (3 of 6 e-matmuls).  Thunking
                chunks at this granularity keeps the per-pair PE load above
                the ACT exp cadence, so the S stream never outruns ACT into
                the 2-slot pss ring."""
                key = (h, n, which)
                w_eh, dst, b_sb = ((wk_eh, kT, bk_sb) if which == "k"
                                   else (wq_eh, qT, bq_sb))
                if part == 0:
                    kq_ps[key] = ps_proj.tile([128, 512], f32_, tag="ps",
                                              name=f"ps{which}{h}_{n}")
                ps = kq_ps[key]
                for e in (2 * part, 2 * part + 1):
                    nc.tensor.matmul(ps,
                                     lhsT=w_eh(e, h),
                                     rhs=xtb[n][:, e * 512:(e + 1) * 512],
                                     start=(e == 0), stop=(e == NE - 1))
                if part == 2:
                    nsl = slice(n * 512, (n + 1) * 512)
                    nc.vector.tensor_scalar_add(dst[h][:, nsl],
                                                kq_ps.pop(key),
                                                b_sb[:, h:h + 1])

            def emit_kq_chunk(h, n, which):
                for part in range(3):
                    emit_kq_part(h, n, which, part)

            def kq_chunks(h):
                for n in range(4):
                    yield ("k", h, n)
                    yield ("q", h, n)

            # ---- output projection chunk (one 128-row q tile) ----
            # Split across two 1-bank psums so it can borrow ps_proj slots;
            # PSUM->SBUF copies go on DVE (ACT is busy with exp here).
            # Tail out-proj chunks, two-phase: head 3's rows live only in
            # packed attnT tile 2, so the t0/t1 matmuls are independent of
            # the final normalization and run DURING its chain; only the t2
            # matmul (accumulation close) waits for the per-qm norm slice.
            # Psums borrow the attention rings (free by then).
            tail_ps = {}

            def emit_out_partial(qm, mode="tail"):
                qsl = slice(qm * 128, (qm + 1) * 128)
                if mode == "win":
                    psA = ps_proj.tile([128, 512], f32_, tag="ps",
                                       name=f"poA{qm}")
                    psB = ps_proj.tile([128, 512], f32_, tag="ps",
                                       name=f"poB{qm}")
                elif mode == "edge":
                    # last-window partial: psA on the ps ring, psB on the
                    # pso ring's spare slot (the previous psof freed once
                    # its normalization drained) so only ONE ps slot is
                    # held across the window/tail boundary
                    psA = ps_proj.tile([128, 512], f32_, tag="ps",
                                       name=f"poA{qm}")
                    psB = ps_o.tile([128, 512], f32_, tag="pso",
                                    name=f"poB{qm}")
                else:
                    # tail: both halves on the pss ring (free then)
                    psA = ps_pair.tile([128, 512], f32_, tag="pss",
                                       name=f"poA{qm}")
                    psB = ps_pair.tile([128, 512], f32_, tag="pss",
                                       name=f"poB{qm}")
                tail_ps[qm] = (psA, psB)
                for t in range(2):
                    nc.tensor.matmul(psA,
                                     lhsT=attnT[t][:, qsl],
                                     rhs=wo_t(t)[:, 0:512],
                                     start=(t == 0), stop=False)
                    nc.tensor.matmul(psB[:, 0:256],
                                     lhsT=attnT[t][:, qsl],
                                     rhs=wo_t(t)[:, 512:768],
                                     start=(t == 0), stop=False)

            def emit_out_finish(qm, windowed=False):
                qsl = slice(qm * 128, (qm + 1) * 128)
                psA, psB = tail_ps.pop(qm)
                nc.tensor.matmul(psA, lhsT=attnT[2][:, qsl],
                                 rhs=wo_t(2)[:, 0:512],
                                 start=False, stop=True)
                nc.tensor.matmul(psB[:, 0:256], lhsT=attnT[2][:, qsl],
                                 rhs=wo_t(2)[:, 512:768],
                                 start=False, stop=True)
                out_sb = outsb.tile([128, EMB], f32_, tag="osb",
                                    name=f"osb{qm}")
                # evictions on DVE in windows (ACT is exp-saturated there;
                # GPSIMD cannot touch PSUM on real hw).  In the tail, psB
                # rides ACT end-to-end (evict + store) so the last stores
                # split across the SP and ACT queues instead of
                # serializing on SP's 650 ns issue cost.
                nc.vector.tensor_copy(out_sb[:, 0:512], psA)
                if windowed:
                    nc.sync.dma_start(
                        out=outp[qm * 128:(qm + 1) * 128, 0:512],
                        in_=out_sb[:, 0:512])
                else:
                    eng = nc.gpsimd if qm % 2 == 0 else nc.sync
                    eng.dma_start(
                        out=outp[qm * 128:(qm + 1) * 128, 0:512],
                        in_=out_sb[:, 0:512])
                if windowed:
                    nc.vector.tensor_copy(out_sb[:, 512:768], psB[:, 0:256])
                    nc.sync.dma_start(
                        out=outp[qm * 128:(qm + 1) * 128, 512:768],
                        in_=out_sb[:, 512:768])
                else:
                    # tail: psB evict on DVE; store rotated across the
                    # ACT and Pool/SWDGE queues so the four final stores
                    # don't serialize on the single HWDGE slot
                    nc.vector.tensor_copy(out_sb[:, 512:768], psB[:, 0:256])
                    eng = nc.scalar if qm % 2 == 0 else nc.gpsimd
                    eng.dma_start(
                        out=outp[qm * 128:(qm + 1) * 128, 512:768],
                        in_=out_sb[:, 512:768])

            # ---- attention emit (with interleaved PE filler work) ----
            # repack: transpose normed[qc][qt] ([128 q, 384 packed head
            # dims]) into attnT via 128x128 PE transposes + DVE evictions.
            # Chunks t0/t1 cover only heads 0-2, so they repack as soon as
            # head 2's normalization lands - during head 3's own windows -
            # and the out-proj partials (which read t0/t1) can then run
            # inside the last window.  Only the t2 chunk (heads 2+3) waits
            # for head 3's normalization.
            def emit_repack01(qc, qt):
                tps = ps_proj.tile([128, 2 * 128], bf, tag="ps",
                                   name=f"tp01_{qc}_{qt}")
                for t_ in range(2):
                    nc.tensor.transpose(
                        tps[:, t_ * 128:(t_ + 1) * 128],
                        normed[qc][qt][:, t_ * 128:(t_ + 1) * 128],
                        ident_sb)
                csl = slice(qc * QC + qt * 128, qc * QC + (qt + 1) * 128)
                for t_ in range(2):
                    nc.vector.tensor_copy(attnT[t_][:, csl],
                                          tps[:, t_ * 128:(t_ + 1) * 128])

            def emit_repack2(qc, qt, tail=False):
                tps = ps_proj.tile([128, 128], bf, tag="ps",
                                   name=f"tp2_{qc}_{qt}")
                nc.tensor.transpose(tps, normed[qc][qt][:, 256:384],
                                    ident_sb)
                csl = slice(qc * QC + qt * 128, qc * QC + (qt + 1) * 128)
                if tail:
                    # keep the tail's DVE queue clear for the out evictions
                    nc.scalar.activation(attnT[2][:, csl], tps,
                                         mybir.ActivationFunctionType.Copy)
                else:
                    nc.vector.tensor_copy(attnT[2][:, csl], tps)

            # cross-window O backlog: each pair's O-block (and, for a
            # window's last pair, its normalization) is queued and drained
            # `lag` pairs later - ACROSS window boundaries for heads 0-2,
            # so a window's trailing O's overlap the next window's S
            # stream.  Head-3 windows drain fully at their end because the
            # following window's repack thunks need the normalization.
            obl = []

            def emit_attention(h, thunks_for_qc):
                """thunks_for_qc(qc) -> list of emit callables injected into
                the PE stream spread across this q-chunk's pairs."""
                for qc in range(NQC):
                    thunks, pos, post = thunks_for_qc(qc)
                    inject_at = {}
                    if pos is not None:
                        for i, t in enumerate(thunks):
                            inject_at.setdefault(
                                pos[i] if i < len(pos) else NPAIR - 1,
                                []).append(t)
                    else:
                        for i, t in enumerate(thunks):
                            inject_at.setdefault(
                                min(i, NPAIR - 1), []).append(t)
                    qsl = slice(qc * QC, (qc + 1) * QC)
                    idx = h * NQC + qc
                    # flipped-O accumulator: [128 q, 4 qt x (96 d + sums)]
                    # packed per q-tile into one psum bank.  The bank holds
                    # FOUR interleaved accumulation chains; hardware supports
                    # only one OPEN start/stop group per bank at a time
                    # (later start=True resets the others), so the tile is
                    # zeroed up front and every matmul accumulates with
                    # start=False.
                    psof = ps_o.tile([128, 4 * HDV], f32_, tag="pso",
                                     name=f"psof{idx}")
                    nc.vector.memset(psof, 0.0)
                    eps = []

                    def emit_ss(p):
                        pss = ps_pair.tile([128, 1024], f32_, tag="pss",
                                           name=f"pss{idx}_{p}")
                        ep = expp.tile([128, 1024], bf, tag="exp",
                                       name=f"exp{idx}_{p}")
                        # exp split: Pool takes the FIRST cols (waits only
                        # on the j=0 matmul, so its q7-launch latency hides
                        # behind the j=1 matmul), ACT takes the rest.  Both
                        # cadences stay under the pair's PE work so PE is
                        # the limiter.  Head 3 (qc>0) keeps exp fully on ACT
                        # - its windows carry out-proj thunks (PE/pair well
                        # above 1038 ns) and Pool absorbs psum evictions.
                        # both S matmuls BEFORE either exp: an exp emitted
                        # between them reads this pss tile and Tile's
                        # conservative subtile tracking then serializes the
                        # second matmul behind it (false intra-tile WAR)
                        nc.tensor.matmul(
                            pss[:, 0:512],
                            lhsT=kT[h][:, (2 * p) * 128:(2 * p + 1) * 128],
                            rhs=qT[h][:, qsl],
                            start=True, stop=True)
                        nc.tensor.matmul(
                            pss[:, 512:1024],
                            lhsT=kT[h][:, (2 * p + 1) * 128:
                                       (2 * p + 2) * 128],
                            rhs=qT[h][:, qsl],
                            start=True, stop=True)
                        nc.scalar.activation(
                            ep, pss,
                            mybir.ActivationFunctionType.Exp,
                            scale=SCALING)
                        eps.append(ep)

                    def emit_o(p, ep, psof=psof, h=h):
                        # flipped: the exp tile is the (free) stationary
                        # load, the 97-col vaug slab streams -> 40 ns per
                        # matmul instead of 213, output lands [q, d]
                        for j in range(2):
                            kt = 2 * p + j
                            for qt in range(4):
                                nc.tensor.matmul(
                                    psof[:, qt * HDV:(qt + 1) * HDV],
                                    lhsT=ep[:, j * 512 + qt * 128:
                                            j * 512 + (qt + 1) * 128],
                                    rhs=vaug[kt][:, h * HDV:(h + 1) * HDV],
                                    start=False, stop=(kt == NKT - 1))

                    def emit_norm(psof=psof, h=h, qc=qc, idx=idx):
                        # sums live at col 96 of each q-tile group; the
                        # reciprocal is a per-partition scalar.  One
                        # contiguous single-column reciprocal per q-tile
                        # (the custom-DVE op mis-lowers strided APs).
                        rbq = rbp.tile([128, 4], f32_, tag="rbq",
                                       name=f"rbq{idx}")
                        for qt in range(4):
                            nc.vector.reciprocal_approx_fast(
                                out=rbq[:, qt:qt + 1],
                                in_=psof[:, qt * HDV + HD:qt * HDV + HD + 1])
                            nc.vector.tensor_scalar_mul(
                                normed[qc][qt][:, h * HD:(h + 1) * HD],
                                psof[:, qt * HDV:qt * HDV + HD],
                                rbq[:, qt:qt + 1])

                    def make_o(p, last, ep):
                        def f():
                            emit_o(p, ep)
                            if last:
                                emit_norm()
                        return f

                    lag = 3 if (h == HPC - 1 and qc == NQC - 1) else 4
                    for p in range(NPAIR):
                        emit_ss(p)
                        for t in inject_at.get(p, ()):
                            t()
                        obl.append(make_o(p, p == NPAIR - 1, eps[-1]))
                        while len(obl) > lag:
                            obl.pop(0)()
                    # drain the O backlog woven with any held-back thunks:
                    # the last pairs' exps finish ~1 us after the S stream,
                    # so the interleaved thunks keep the PE busy across
                    # that latency and the following norm chain
                    post = list(post)
                    while obl:
                        obl.pop(0)()
                        if len(obl) <= 2 and post:
                            post.pop(0)()
                    while post:
                        post.pop(0)()

            # ---- emission schedule ----
            # consume each x seq-block as its DMA lands: head 0's k/q chunks
            # for block n, then the V chunks of block n
            for n in range(4):
                emit_kq_chunk(0, n, "k")
                emit_kq_chunk(0, n, "q")
                for kt in range(4 * n, 4 * n + 4):
                    emit_v_chunk(kt)

            def kq_thunks(hnext):
                def f(qc):
                    # 2 chunks per q-chunk, split into 3-matmul halves (4
                    # thunks of ~640 ns) so the PE filler spreads across the
                    # window instead of lumping at two pairs
                    items = list(kq_chunks(hnext))[2 * qc:2 * qc + 2]
                    out = []
                    for it in items:
                        for part in range(3):
                            out.append(
                                lambda it=it, part=part: emit_kq_part(
                                    it[1], it[2], it[0], part))
                    return out, None, []
                return f

            def rp01_thunks(qc):
                return [lambda qt=qt: emit_repack01(qc, qt)
                        for qt in range(4)]

            def out_thunks(qc):
                # head-3 window qc: close out q-chunk qc-1 (t2 repack + P/F
                # two-phase out-proj) and pre-repack t0/t1 of q-chunk qc+1
                # (they only need heads 0-2, normalized long ago).  The
                # last window also starts P(q0+0) with its psB on the pso
                # spare slot so the tail begins with a partial in flight.
                if qc == 0:
                    return (rp01_thunks(0) + rp01_thunks(1)
                            + rp01_thunks(2),
                            [0, 1, 1, 2, 3, 3, 4, 5, 5, 6, 7, 7], [])
                qp = qc - 1
                out = []
                for qt4 in range(0, 4, 2):
                    out.append(lambda qt=qt4: emit_repack2(qp, qt))
                    out.append(lambda qt=qt4 + 1: emit_repack2(qp, qt))
                    for qm in (4 * qp + qt4, 4 * qp + qt4 + 1):
                        out.append(
                            lambda qm=qm: emit_out_partial(qm, "win"))
                        out.append(lambda qm=qm: emit_out_finish(qm, True))
                pos = [0, 0, 1, 1, 2, 2, 3, 3, 4, 4, 5, 5]
                if qc == NQC - 2:
                    out += rp01_thunks(qc + 1)
                    pos += [6, 6, 7, 7]
                elif qc == NQC - 1:
                    out.append(lambda: emit_out_partial(4 * qc, "edge"))
                    pos += [7]
                return out, pos, []

            for h in range(HPC - 1):
                emit_attention(h, kq_thunks(h + 1))
            emit_attention(HPC - 1, out_thunks)
            # tail: only the t2 repacks and the finish halves remain; each
            # F(qm) frees the ps/pso slots its successor partial needs
            q0 = 4 * (NQC - 1)
            emit_repack2(NQC - 1, 0, True)
            emit_out_finish(q0)
            emit_out_partial(q0 + 1)
            emit_repack2(NQC - 1, 1, True)
            emit_out_finish(q0 + 1)
            emit_out_partial(q0 + 2)
            emit_repack2(NQC - 1, 2, True)
            emit_out_finish(q0 + 2)
            emit_out_partial(q0 + 3)
            emit_repack2(NQC - 1, 3, True)
            emit_out_finish(q0 + 3)

            if os.environ.get("KDEBUG"):
                dbg_attn = nc.dram_tensor(
                    "dbg_attn", [3 * 128, SEQ], bf,
                    kind="ExternalOutput").ap()
                for t_ in range(3):
                    nc.sync.dma_start(
                        out=dbg_attn[t_ * 128:(t_ + 1) * 128, :],
                        in_=attnT[t_])
                dbg_nm = nc.dram_tensor(
                    "dbg_nm", [NQC * 4 * 128, HPC * HD], bf,
                    kind="ExternalOutput").ap()
                for qc in range(NQC):
                    for qt in range(4):
                        r0 = (qc * 4 + qt) * 128
                        nc.sync.dma_start(out=dbg_nm[r0:r0 + 128, :],
                                          in_=normed[qc][qt])

    nc.compile()
    return nc


def _get_nc():
    if "nc" not in _NC_CACHE:
        _NC_CACHE["nc"] = _build_nc()
    return _NC_CACHE["nc"]


def _etile_pack(wT):
    """[768, n] (e on rows) -> [128, 6*n] bf16: e-tiles as column blocks so
    the whole operand loads as ONE [128, n] DMA."""
    n = wT.shape[1]
    a = wT.reshape(NE, 128, n).transpose(1, 0, 2)
    return np.ascontiguousarray(a.reshape(128, NE * n)).astype(BF16)


def _x_block_pack(x_b):
    """[2048, 768] x -> [128, 4 * 6 * 512] bf16, seq-block-major: block n
    holds e-tiles of sequence rows n*512..(n+1)*512 as column slabs."""
    a = x_b.reshape(4, 512, NE, 128)          # n, s, e, p
    a = a.transpose(3, 0, 2, 1)               # p, n, e, s
    return np.ascontiguousarray(a.reshape(128, 4 * NE * 512)).astype(BF16)


def _pad_headsT(w_rows):
    """[384, 768] head rows -> zero-pad head dim 96->128 -> transpose -> [768, 512]."""
    p = np.zeros((HPC * HDP, EMB), np.float32)
    p.reshape(HPC, HDP, EMB)[:, :HD] = w_rows.reshape(HPC, HD, EMB)
    return np.ascontiguousarray(p.T)


def _pad_bias(b_rows):
    """[384] head bias -> [128, HPC] padded/transposed for per-partition add."""
    p = np.zeros((HPC, HDP), np.float32)
    p[:, :HD] = b_rows.reshape(HPC, HD)
    return np.ascontiguousarray(p.T)


def kernel(x, Wq, bq, Wk, bk, Wv, bv, Wo, bo):
    x = np.asarray(x, np.float32)
    Wq, bq = np.asarray(Wq, np.float32), np.asarray(bq, np.float32)
    Wk, bk = np.asarray(Wk, np.float32), np.asarray(bk, np.float32)
    Wv, bv = np.asarray(Wv, np.float32), np.asarray(bv, np.float32)
    Wo, bo = np.asarray(Wo, np.float32), np.asarray(bo, np.float32)

    nc = _get_nc()

    in_maps = []
    for c in range(NCORES):
        b, hg = divmod(c, 2)
        hs = slice(hg * HPC * HD, (hg + 1) * HPC * HD)
        woT = Wo[:, hs].T  # [384, 768]
        wo_pack = np.ascontiguousarray(
            woT.reshape(3, 128, EMB).transpose(1, 0, 2).reshape(128, 3 * EMB))
        wq_et = _etile_pack(_pad_headsT(Wq[hs])).reshape(128, NE, HPC, HDP)
        wk_et = _etile_pack(_pad_headsT(Wk[hs])).reshape(128, NE, HPC, HDP)
        in_maps.append({
            "xtp": _x_block_pack(x[b]),
            "wqp0": np.ascontiguousarray(
                wq_et[:, :, 0].reshape(128, NE * HDP)),
            "wqpr": np.ascontiguousarray(
                wq_et[:, :, 1:].reshape(128, NE * 3 * HDP)),
            "wkp0": np.ascontiguousarray(
                wk_et[:, :, 0].reshape(128, NE * HDP)),
            "wkpr": np.ascontiguousarray(
                wk_et[:, :, 1:].reshape(128, NE * 3 * HDP)),
            "wvp": _etile_pack(np.ascontiguousarray(Wv[hs].T)),
            "wop": wo_pack.astype(BF16),
            "bqp": _pad_bias(bq[hs]),
            "bkp": _pad_bias(bk[hs]),
            "identp": np.ascontiguousarray(np.eye(128, dtype=np.float32))
            .astype(BF16),
        })

    global LAST_RESULT
    trace = bool(int(os.environ.get("KERNEL_TRACE", "0")))
    tmpdir = os.environ.get("KERNEL_TRACE_DIR") or None
    res = run_bass_kernel_spmd(nc, in_maps, list(range(NCORES)), trace=trace,
                               tmpdir=tmpdir)
    LAST_RESULT = res

    out = np.empty((B, SEQ, EMB), np.float32)
    for b in range(B):
        out[b] = res.results[2 * b]["outp"] + res.results[2 * b + 1]["outp"]
    # bv enters each head's output additively (sum of softmax weights is 1),
    # and bo is a plain add: both fold into one constant vector.
    out += Wo @ bv + bo
    return out



# revision 144
# speedup vs baseline: 1.0060x; 1.0007x over previous
"""Multi-head attention (B=4, S=2048, E=768, H=8, D=96) on 8 Trainium2 cores.

Sharding: core c -> (batch b = c//2, head-group hg = c%2 of 4 heads).
Each core computes Q/K/V projections for its 4 heads over the full sequence
of its batch, full attention for those heads, and a partial output
projection (row-split Wo).  The two cores of a batch produce partial
outputs that are summed on the host during unsharding (tensor-parallel
reduce).

On-chip layout notes:
  - All matmul operands are bf16 (1 cycle/row on PE; fp32 would be 4x; fp8
    DoubleRow would halve PE time but its ~3% per-element noise lands ~1:1
    in the output and busts the 2e-2 tolerance).
  - Scores are computed transposed, S^T[k, q] = K^T.T @ Q^T (the K/Q
    projections pad head dim 96->128 on the contraction partitions, which
    costs nothing).  exp(S) runs on ACT straight out of PSUM over a
    [128, 1024] pair of key tiles with the 1/sqrt(d) scale folded in.
    ACT's 1038 ns per pair is the cadence ceiling for bare windows.
  - The O = V^T exp(S) matmul is FLIPPED: the exp tile is the stationary
    operand (Ldweights is free) and a 97-column vaug slab (96 V dims + a
    ones column) streams, so each matmul costs 40 ns instead of 213 and
    the per-pair PE work drops from 852 to 749 ns.  The ones column makes
    the softmax denominator fall out at column 96 of each q-tile group of
    the [q, 4 x 97] accumulator, so normalization is a per-partition
    reciprocal + tensor_scalar - no broadcast machinery at all.
  - A PSUM bank supports only ONE open accumulation group at a time, so
    the flipped accumulator (4 interleaved chains in one bank) is zeroed
    up front and every matmul accumulates with start=False.
  - The normalized [q, d] output is transposed back to the [d, q] layout
    the output projection needs via 128x128 PE transposes (53 ns each)
    against a host-provided identity.  attnT chunks t0/t1 only contain
    heads 0-2, so they repack during head 3's own windows (the ACT-bound
    first window absorbs three q-chunks' worth as PE filler); only the t2
    chunk waits for head 3's normalization, which keeps the tail short.
  - O matmuls consume exps with a 4-pair lag so the ACT exp latency
    (~1.35 us from S matmul to usable exp) never stalls the PE.  Both S
    matmuls of a pair are emitted BEFORE the exp that reads them: Tile's
    conservative subtile tracking would otherwise serialize the second
    matmul behind the exp (false intra-tile WAR).
  - Inputs are host-packed so each operand is ONE [128, n] DMA.  x is
    packed seq-block-major; the first x half rides the Pool/SWDGE queue
    (independent of the HWDGE slot) so it overlaps the wk0 load; head 0's
    K/Q weight columns load before the other heads'.
  - PE stream order: head 0's K/Q projections and V chunks per x block,
    then 16 attention windows (4 heads x 4 q-chunks).  Heads 0-2 carry
    the next head's K/Q chunks as thunked PE filler; head 3's windows
    carry the previous q-chunk's t2 repack + two-phase output projection
    (P = t0/t1 partials, F = t2 close + DVE evictions + stores split
    across the SP/ACT/SWDGE queues).  PSUM rings: pss pairs (2x2 banks),
    flipped-O accumulators (2x1), projection/repack ring (2x1); every
    allocation is placed one eviction behind its ring slot's previous
    user.
"""

import os
import sys

sys.path.insert(0, "/opt/trn_rl_repo")

import numpy as np
import ml_dtypes

import concourse.bacc as bacc
import concourse.tile as tile
from concourse import mybir
from concourse.bass_utils import run_bass_kernel_spmd

BF16 = ml_dtypes.bfloat16

EMB = 768
HEADS = 8
HD = 96          # true head dim
HDP = 128        # padded head dim
SEQ = 2048
B = 4
NCORES = 8
HPC = 4          # heads per core
SCALING = HD ** -0.5
QC = 512         # query chunk per attention inner loop
NQC = SEQ // QC
NKT = SEQ // 128  # 16 key tiles
NPAIR = NKT // 2
NE = EMB // 128   # 6 e_in tiles

_NC_CACHE = {}
LAST_RESULT = None  # BassKernelResults of the most recent run (for test.py)


def _build_nc():
    f32 = mybir.dt.float32
    bf = mybir.dt.bfloat16

    nc = bacc.Bacc(trn_type="TRN2", target_bir_lowering=False, debug=False,
                   num_devices=NCORES)

    # All operands host-packed into [128, n] so each loads as ONE DMA.
    xtp = nc.dram_tensor("xtp", [128, NE * SEQ], bf, kind="ExternalInput").ap()
    # K/Q weights split: head 0's columns load first (0.19MB) so head 0's
    # projections - which gate the whole pipeline - start ~5 us earlier
    wqp0 = nc.dram_tensor("wqp0", [128, NE * HDP], bf,
                          kind="ExternalInput").ap()
    wqpr = nc.dram_tensor("wqpr", [128, NE * 3 * HDP], bf,
                          kind="ExternalInput").ap()
    wkp0 = nc.dram_tensor("wkp0", [128, NE * HDP], bf,
                          kind="ExternalInput").ap()
    wkpr = nc.dram_tensor("wkpr", [128, NE * 3 * HDP], bf,
                          kind="ExternalInput").ap()
    wvp = nc.dram_tensor("wvp", [128, NE * HPC * HD], bf,
                         kind="ExternalInput").ap()
    wop = nc.dram_tensor("wop", [128, 3 * EMB], bf, kind="ExternalInput").ap()
    bqp = nc.dram_tensor("bqp", [128, HPC], f32, kind="ExternalInput").ap()
    bkp = nc.dram_tensor("bkp", [128, HPC], f32, kind="ExternalInput").ap()
    identp = nc.dram_tensor("identp", [128, 128], bf,
                            kind="ExternalInput").ap()
    outp = nc.dram_tensor("outp", [SEQ, EMB], f32, kind="ExternalOutput").ap()

    with tile.TileContext(nc) as tc:
        with (
            tc.tile_pool(name="const", bufs=1) as constp,
            tc.tile_pool(name="big", bufs=1) as bigp,
            tc.tile_pool(name="expp", bufs=6) as expp,
            tc.tile_pool(name="rbp", bufs=4) as rbp,
            tc.tile_pool(name="normp", bufs=1) as normp,
            tc.tile_pool(name="outsb", bufs=12) as outsb,
            tc.tile_pool(name="ps_proj", bufs=2, space="PSUM") as ps_proj,
            tc.tile_pool(name="ps_o", bufs=2, space="PSUM") as ps_o,
            tc.tile_pool(name="ps_pair", bufs=2, space="PSUM") as ps_pair,
        ):
            # ---- loads. x is packed seq-block-major ([128, 6e x 512] per
            # 512-sequence block) so the first K/Q chunk only needs wk + one
            # 0.75MB block; wk + block 0/1 load first, k/q chunks of head 0
            # then pipeline behind the remaining block DMAs. ----
            XB = NE * 512  # 3072 cols per seq block
            wk0_sb = constp.tile([128, NE * HDP], bf, name="wk0_sb")
            nc.sync.dma_start(out=wk0_sb, in_=wkp0)
            xtb = [bigp.tile([128, XB], bf, name=f"xtb{n}") for n in range(4)]
            # the first x half rides the Pool/SWDGE queue: its descriptor
            # path is independent of the HWDGE slot the weight loads use,
            # so it overlaps wk0 end-to-end
            nc.gpsimd.dma_start(out=xtb[0][:, 0:XB // 2],
                                in_=xtp[:, 0:XB // 2])
            nc.sync.dma_start(out=xtb[0][:, XB // 2:XB],
                              in_=xtp[:, XB // 2:XB])
            # biases next: tiny, but they gate the K/Q psum evictions
            bq_sb = constp.tile([128, HPC], f32, name="bq_sb")
            nc.sync.dma_start(out=bq_sb, in_=bqp)
            bk_sb = constp.tile([128, HPC], f32, name="bk_sb")
            nc.sync.dma_start(out=bk_sb, in_=bkp)
            wq0_sb = constp.tile([128, NE * HDP], bf, name="wq0_sb")
            nc.sync.dma_start(out=wq0_sb, in_=wqp0)
            wv_sb = constp.tile([128, NE * HPC * HD], bf, name="wv_sb")
            nc.sync.dma_start(out=wv_sb, in_=wvp)
            nc.sync.dma_start(out=xtb[1], in_=xtp[:, XB:2 * XB])
            wkr_sb = constp.tile([128, NE * 3 * HDP], bf, name="wkr_sb")
            nc.sync.dma_start(out=wkr_sb, in_=wkpr)
            wqr_sb = constp.tile([128, NE * 3 * HDP], bf, name="wqr_sb")
            nc.sync.dma_start(out=wqr_sb, in_=wqpr)
            nc.sync.dma_start(out=xtb[2], in_=xtp[:, 2 * XB:3 * XB])
            nc.sync.dma_start(out=xtb[3], in_=xtp[:, 3 * XB:4 * XB])
            wo_sb = constp.tile([128, 3 * EMB], bf, name="wo_sb")
            nc.sync.dma_start(out=wo_sb, in_=wop)
            # identity for the PE repack transposes (first needed in head
            # 3's phase, so it loads last)
            ident_sb = constp.tile([128, 128], bf, name="ident_sb")
            nc.sync.dma_start(out=ident_sb, in_=identp)

            def wk_eh(e, h):
                if h == 0:
                    return wk0_sb[:, e * HDP:(e + 1) * HDP]
                return wkr_sb[:, (e * 3 + h - 1) * HDP:(e * 3 + h) * HDP]

            def wq_eh(e, h):
                if h == 0:
                    return wq0_sb[:, e * HDP:(e + 1) * HDP]
                return wqr_sb[:, (e * 3 + h - 1) * HDP:(e * 3 + h) * HDP]

            def wv_e(e):
                return wv_sb[:, e * HPC * HD:(e + 1) * HPC * HD]

            def wo_t(t_):
                return wo_sb[:, t_ * EMB:(t_ + 1) * EMB]

            # ---- persistent intermediates ----
            # vaug: per key tile, 4 heads x (96 v-cols + a ones col).  The
            # ones col makes row^T @ vaug yield the softmax denominator in
            # the same accumulator (col 96 of each head group).  No pad
            # cols: vaug is the MOVING operand of the flipped O matmul, so
            # narrower means cheaper (97 vs 128 cycles).
            HDV = HD + 1
            vaug = []
            for kt in range(NKT):
                t = bigp.tile([128, HPC * HDV], bf, name=f"vaug{kt}")
                ones_cols = t.rearrange("p (h c) -> p h c",
                                        h=HPC)[:, :, HD:HD + 1]
                nc.gpsimd.memset(ones_cols, 1.0)
                vaug.append(t)
            qT = [bigp.tile([128, SEQ], bf, name=f"qT{h}") for h in range(HPC)]
            kT = [bigp.tile([128, SEQ], bf, name=f"kT{h}") for h in range(HPC)]
            # packed attention output, [384 rows = 3 tiles x 128, seq],
            # written by the h3-phase repack transposes
            attnT = [bigp.tile([128, SEQ], bf, name=f"attnT{t_}")
                     for t_ in range(3)]
            # normalized O in [q, head-dims] layout, per (q-chunk, q-tile):
            # written per head as its window completes, transposed into
            # attnT once all four heads are in
            normed = [[normp.tile([128, HPC * HD], bf, name=f"nm{qc}_{qt}")
                       for qt in range(4)] for qc in range(NQC)]

            f32_ = f32

            # ---- projection emit helpers ----
            def emit_v_chunk(kt):
                psv = ps_proj.tile([128, 512], f32_, tag="ps",
                                   name=f"psv{kt}")
                blk, off = divmod(kt, 4)
                for e in range(NE):
                    nc.tensor.matmul(psv[:, 0:HPC * HD],
                                     lhsT=xtb[blk][:, e * 512 + off * 128:
                                                   e * 512 + off * 128 + 128],
                                     rhs=wv_e(e),
                                     start=(e == 0), stop=(e == NE - 1))
                # one strided copy for all four heads: 525 ns instead of
                # 4 x 225, and the ps-ring slot frees a full copy earlier
                nc.vector.tensor_copy(
                    vaug[kt].rearrange("p (h c) -> p h c",
                                       h=HPC)[:, :, 0:HD],
                    psv[:, 0:HPC * HD].rearrange("p (h c) -> p h c",
                                                 h=HPC))

            kq_ps = {}

            def emit_kq_part(h, n, which, part):
                """One half of a K/Q chunk (3 of 6 e-matmuls).  Thunking
                chunks at this granularity keeps the per-pair PE load above
                the ACT exp cadence, so the S stream never outruns ACT into
                the 2-slot pss ring."""
                key = (h, n, which)
                w_eh, dst, b_sb = ((wk_eh, kT, bk_sb) if which == "k"
                                   else (wq_eh, qT, bq_sb))
                if part == 0:
                    kq_ps[key] = ps_proj.tile([128, 512], f32_, tag="ps",
                                              name=f"ps{which}{h}_{n}")
                ps = kq_ps[key]
                for e in (2 * part, 2 * part + 1):
                    nc.tensor.matmul(ps,
                                     lhsT=w_eh(e, h),
                                     rhs=xtb[n][:, e * 512:(e + 1) * 512],
                                     start=(e == 0), stop=(e == NE - 1))
                if part == 2:
                    nsl = slice(n * 512, (n + 1) * 512)
                    nc.vector.tensor_scalar_add(dst[h][:, nsl],
                                                kq_ps.pop(key),
                                                b_sb[:, h:h + 1])

            def emit_kq_chunk(h, n, which):
                for part in range(3):
                    emit_kq_part(h, n, which, part)

            def kq_chunks(h):
                for n in range(4):
                    yield ("k", h, n)
                    yield ("q", h, n)

            # ---- output projection chunk (one 128-row q tile) ----
            # Split across two 1-bank psums so it can borrow ps_proj slots;
            # PSUM->SBUF copies go on DVE (ACT is busy with exp here).
            # Tail out-proj chunks, two-phase: head 3's rows live only in
            # packed attnT tile 2, so the t0/t1 matmuls are independent of
            # the final normalization and run DURING its chain; only the t2
            # matmul (accumulation close) waits for the per-qm norm slice.
            # Psums borrow the attention rings (free by then).
            tail_ps = {}

            def emit_out_partial(qm, mode="tail"):
                qsl = slice(qm * 128, (qm + 1) * 128)
                if mode == "win":
                    psA = ps_proj.tile([128, 512], f32_, tag="ps",
                                       name=f"poA{qm}")
                    psB = ps_proj.tile([128, 512], f32_, tag="ps",
                                       name=f"poB{qm}")
                elif mode == "edge":
                    # last-window partial: psA on the ps ring, psB on the
                    # pso ring's spare slot (the previous psof freed once
                    # its normalization drained) so only ONE ps slot is
                    # held across the window/tail boundary
                    psA = ps_proj.tile([128, 512], f32_, tag="ps",
                                       name=f"poA{qm}")
                    psB = ps_o.tile([128, 512], f32_, tag="pso",
                                    name=f"poB{qm}")
                else:
                    # tail: both halves on the pss ring (free then)
                    psA = ps_pair.tile([128, 512], f32_, tag="pss",
                                       name=f"poA{qm}")
                    psB = ps_pair.tile([128, 512], f32_, tag="pss",
                                       name=f"poB{qm}")
                tail_ps[qm] = (psA, psB)
                for t in range(2):
                    nc.tensor.matmul(psA,
                                     lhsT=attnT[t][:, qsl],
                                     rhs=wo_t(t)[:, 0:512],
                                     start=(t == 0), stop=False)
                    nc.tensor.matmul(psB[:, 0:256],
                                     lhsT=attnT[t][:, qsl],
                                     rhs=wo_t(t)[:, 512:768],
                                     start=(t == 0), stop=False)

            def emit_out_finish(qm, windowed=False):
                qsl = slice(qm * 128, (qm + 1) * 128)
                psA, psB = tail_ps.pop(qm)
                nc.tensor.matmul(psA, lhsT=attnT[2][:, qsl],
                                 rhs=wo_t(2)[:, 0:512],
                                 start=False, stop=True)
                nc.tensor.matmul(psB[:, 0:256], lhsT=attnT[2][:, qsl],
                                 rhs=wo_t(2)[:, 512:768],
                                 start=False, stop=True)
                out_sb = outsb.tile([128, EMB], f32_, tag="osb",
                                    name=f"osb{qm}")
                # evictions on DVE in windows (ACT is exp-saturated there;
                # GPSIMD cannot touch PSUM on real hw).  In the tail, psB
                # rides ACT end-to-end (evict + store) so the last stores
                # split across the SP and ACT queues instead of
                # serializing on SP's 650 ns issue cost.
                nc.vector.tensor_copy(out_sb[:, 0:512], psA)
                if windowed:
                    nc.sync.dma_start(
                        out=outp[qm * 128:(qm + 1) * 128, 0:512],
                        in_=out_sb[:, 0:512])
                else:
                    eng = nc.gpsimd if qm % 2 == 0 else nc.sync
                    eng.dma_start(
                        out=outp[qm * 128:(qm + 1) * 128, 0:512],
                        in_=out_sb[:, 0:512])
                if windowed:
                    nc.vector.tensor_copy(out_sb[:, 512:768], psB[:, 0:256])
                    nc.sync.dma_start(
                        out=outp[qm * 128:(qm + 1) * 128, 512:768],
                        in_=out_sb[:, 512:768])
                else:
                    # tail: psB evict on DVE; store rotated across the
                    # ACT and Pool/SWDGE queues so the four final stores
                    # don't serialize on the single HWDGE slot
                    nc.vector.tensor_copy(out_sb[:, 512:768], psB[:, 0:256])
                    eng = nc.scalar if qm % 2 == 0 else nc.gpsimd
                    eng.dma_start(
                        out=outp[qm * 128:(qm + 1) * 128, 512:768],
                        in_=out_sb[:, 512:768])

            # ---- attention emit (with interleaved PE filler work) ----
            # repack: transpose normed[qc][qt] ([128 q, 384 packed head
            # dims]) into attnT via 128x128 PE transposes + DVE evictions.
            # Chunks t0/t1 cover only heads 0-2, so they repack as soon as
            # head 2's normalization lands - during head 3's own windows -
            # and the out-proj partials (which read t0/t1) can then run
            # inside the last window.  Only the t2 chunk (heads 2+3) waits
            # for head 3's normalization.
            def emit_repack01(qc, qt):
                tps = ps_proj.tile([128, 2 * 128], bf, tag="ps",
                                   name=f"tp01_{qc}_{qt}")
                for t_ in range(2):
                    nc.tensor.transpose(
                        tps[:, t_ * 128:(t_ + 1) * 128],
                        normed[qc][qt][:, t_ * 128:(t_ + 1) * 128],
                        ident_sb)
                csl = slice(qc * QC + qt * 128, qc * QC + (qt + 1) * 128)
                for t_ in range(2):
                    nc.vector.tensor_copy(attnT[t_][:, csl],
                                          tps[:, t_ * 128:(t_ + 1) * 128])

            def emit_repack2(qc, qt, tail=False):
                tps = ps_proj.tile([128, 128], bf, tag="ps",
                                   name=f"tp2_{qc}_{qt}")
                nc.tensor.transpose(tps, normed[qc][qt][:, 256:384],
                                    ident_sb)
                csl = slice(qc * QC + qt * 128, qc * QC + (qt + 1) * 128)
                if tail:
                    # keep the tail's DVE queue clear for the out evictions
                    nc.scalar.activation(attnT[2][:, csl], tps,
                                         mybir.ActivationFunctionType.Copy)
                else:
                    nc.vector.tensor_copy(attnT[2][:, csl], tps)

            # cross-window O backlog: each pair's O-block (and, for a
            # window's last pair, its normalization) is queued and drained
            # `lag` pairs later - ACROSS window boundaries for heads 0-2,
            # so a window's trailing O's overlap the next window's S
            # stream.  Head-3 windows drain fully at their end because the
            # following window's repack thunks need the normalization.
            obl = []

            def emit_attention(h, thunks_for_qc):
                """thunks_for_qc(qc) -> list of emit callables injected into
                the PE stream spread across this q-chunk's pairs."""
                for qc in range(NQC):
                    thunks, pos, post = thunks_for_qc(qc)
                    inject_at = {}
                    if pos is not None:
                        for i, t in enumerate(thunks):
                            inject_at.setdefault(
                                pos[i] if i < len(pos) else NPAIR - 1,
                                []).append(t)
                    else:
                        for i, t in enumerate(thunks):
                            inject_at.setdefault(
                                min(i, NPAIR - 1), []).append(t)
                    qsl = slice(qc * QC, (qc + 1) * QC)
                    idx = h * NQC + qc
                    # flipped-O accumulator: [128 q, 4 qt x (96 d + sums)]
                    # packed per q-tile into one psum bank.  The bank holds
                    # FOUR interleaved accumulation chains; hardware supports
                    # only one OPEN start/stop group per bank at a time
                    # (later start=True resets the others), so the tile is
                    # zeroed up front and every matmul accumulates with
                    # start=False.
                    psof = ps_o.tile([128, 4 * HDV], f32_, tag="pso",
                                     name=f"psof{idx}")
                    nc.vector.memset(psof, 0.0)
                    eps = []

                    def emit_ss(p):
                        pss = ps_pair.tile([128, 1024], f32_, tag="pss",
                                           name=f"pss{idx}_{p}")
                        ep = expp.tile([128, 1024], bf, tag="exp",
                                       name=f"exp{idx}_{p}")
                        # exp split: Pool takes the FIRST cols (waits only
                        # on the j=0 matmul, so its q7-launch latency hides
                        # behind the j=1 matmul), ACT takes the rest.  Both
                        # cadences stay under the pair's PE work so PE is
                        # the limiter.  Head 3 (qc>0) keeps exp fully on ACT
                        # - its windows carry out-proj thunks (PE/pair well
                        # above 1038 ns) and Pool absorbs psum evictions.
                        # both S matmuls BEFORE either exp: an exp emitted
                        # between them reads this pss tile and Tile's
                        # conservative subtile tracking then serializes the
                        # second matmul behind it (false intra-tile WAR)
                        nc.tensor.matmul(
                            pss[:, 0:512],
                            lhsT=kT[h][:, (2 * p) * 128:(2 * p + 1) * 128],
                            rhs=qT[h][:, qsl],
                            start=True, stop=True)
                        nc.tensor.matmul(
                            pss[:, 512:1024],
                            lhsT=kT[h][:, (2 * p + 1) * 128:
                                       (2 * p + 2) * 128],
                            rhs=qT[h][:, qsl],
                            start=True, stop=True)
                        nc.scalar.activation(
                            ep, pss,
                            mybir.ActivationFunctionType.Exp,
                            scale=SCALING)
                        eps.append(ep)

                    def emit_o(p, ep, psof=psof, h=h):
                        # flipped: the exp tile is the (free) stationary
                        # load, the 97-col vaug slab streams -> 40 ns per
                        # matmul instead of 213, output lands [q, d]
                        for j in range(2):
                            kt = 2 * p + j
                            for qt in range(4):
                                nc.tensor.matmul(
                                    psof[:, qt * HDV:(qt + 1) * HDV],
                                    lhsT=ep[:, j * 512 + qt * 128:
                                            j * 512 + (qt + 1) * 128],
                                    rhs=vaug[kt][:, h * HDV:(h + 1) * HDV],
                                    start=False, stop=(kt == NKT - 1))

                    def emit_norm(psof=psof, h=h, qc=qc, idx=idx):
                        # sums live at col 96 of each q-tile group; the
                        # reciprocal is a per-partition scalar.  One
                        # contiguous single-column reciprocal per q-tile
                        # (the custom-DVE op mis-lowers strided APs).
                        rbq = rbp.tile([128, 4], f32_, tag="rbq",
                                       name=f"rbq{idx}")
                        for qt in range(4):
                            nc.vector.reciprocal_approx_fast(
                                out=rbq[:, qt:qt + 1],
                                in_=psof[:, qt * HDV + HD:qt * HDV + HD + 1])
                            nc.vector.tensor_scalar_mul(
                                normed[qc][qt][:, h * HD:(h + 1) * HD],
                                psof[:, qt * HDV:qt * HDV + HD],
                                rbq[:, qt:qt + 1])

                    def make_o(p, last, ep):
                        def f():
                            emit_o(p, ep)
                            if last:
                                emit_norm()
                        return f

                    lag = 3 if (h == HPC - 1 and qc == NQC - 1) else 4
                    for p in range(NPAIR):
                        emit_ss(p)
                        for t in inject_at.get(p, ()):
                            t()
                        obl.append(make_o(p, p == NPAIR - 1, eps[-1]))
                        while len(obl) > lag:
                            obl.pop(0)()
                    # drain the O backlog woven with any held-back thunks:
                    # the last pairs' exps finish ~1 us after the S stream,
                    # so the interleaved thunks keep the PE busy across
                    # that latency and the following norm chain
                    post = list(post)
                    while obl:
                        obl.pop(0)()
                        if len(obl) <= 2 and post:
                            post.pop(0)()
                    while post:
                        post.pop(0)()

            # ---- emission schedule ----
            # consume each x seq-block as its DMA lands: head 0's k/q chunks
            # for block n, then the V chunks of block n
            for n in range(4):
                emit_kq_chunk(0, n, "k")
                emit_kq_chunk(0, n, "q")
                for kt in range(4 * n, 4 * n + 4):
                    emit_v_chunk(kt)

            def kq_thunks(hnext):
                def f(qc):
                    # 2 chunks per q-chunk, split into 3-matmul halves (4
                    # thunks of ~640 ns) so the PE filler spreads across the
                    # window instead of lumping at two pairs
                    items = list(kq_chunks(hnext))[2 * qc:2 * qc + 2]
                    out = []
                    for it in items:
                        for part in range(3):
                            out.append(
                                lambda it=it, part=part: emit_kq_part(
                                    it[1], it[2], it[0], part))
                    return out, None, []
                return f

            def rp01_thunks(qc):
                return [lambda qt=qt: emit_repack01(qc, qt)
                        for qt in range(4)]

            def out_thunks(qc):
                # head-3 window qc: close out q-chunk qc-1 (t2 repack + P/F
                # two-phase out-proj) and pre-repack t0/t1 of q-chunk qc+1
                # (they only need heads 0-2, normalized long ago).  The
                # last window also starts P(q0+0) with its psB on the pso
                # spare slot so the tail begins with a partial in flight.
                if qc == 0:
                    return (rp01_thunks(0) + rp01_thunks(1)
                            + rp01_thunks(2) + rp01_thunks(3),
                            [0, 0, 1, 1, 2, 2, 3, 3, 4, 4, 5, 5,
                             6, 6, 7, 7], [])
                qp = qc - 1
                out = []
                for qt4 in range(0, 4, 2):
                    out.append(lambda qt=qt4: emit_repack2(qp, qt))
                    out.append(lambda qt=qt4 + 1: emit_repack2(qp, qt))
                    for qm in (4 * qp + qt4, 4 * qp + qt4 + 1):
                        out.append(
                            lambda qm=qm: emit_out_partial(qm, "win"))
                        out.append(lambda qm=qm: emit_out_finish(qm, True))
                pos = [0, 0, 1, 1, 2, 2, 3, 3, 4, 4, 5, 5]
                if qc == NQC - 1:
                    out.append(lambda: emit_out_partial(4 * qc, "edge"))
                    pos += [7]
                return out, pos, []

            for h in range(HPC - 1):
                emit_attention(h, kq_thunks(h + 1))
            emit_attention(HPC - 1, out_thunks)
            # tail: only the t2 repacks and the finish halves remain; each
            # F(qm) frees the ps/pso slots its successor partial needs
            q0 = 4 * (NQC - 1)
            emit_repack2(NQC - 1, 0, True)
            emit_out_finish(q0)
            emit_out_partial(q0 + 1)
            emit_repack2(NQC - 1, 1, True)
            emit_out_finish(q0 + 1)
            emit_out_partial(q0 + 2)
            emit_repack2(NQC - 1, 2, True)
            emit_out_finish(q0 + 2)
            emit_out_partial(q0 + 3)
            emit_repack2(NQC - 1, 3, True)
            emit_out_finish(q0 + 3)

            if os.environ.get("KDEBUG"):
                dbg_attn = nc.dram_tensor(
                    "dbg_attn", [3 * 128, SEQ], bf,
                    kind="ExternalOutput").ap()
                for t_ in range(3):
                    nc.sync.dma_start(
                        out=dbg_attn[t_ * 128:(t_ + 1) * 128, :],
                        in_=attnT[t_])
                dbg_nm = nc.dram_tensor(
                    "dbg_nm", [NQC * 4 * 128, HPC * HD], bf,
                    kind="ExternalOutput").ap()
                for qc in range(NQC):
                    for qt in range(4):
                        r0 = (qc * 4 + qt) * 128
                        nc.sync.dma_start(out=dbg_nm[r0:r0 + 128, :],
                                          in_=normed[qc][qt])

    nc.compile()
    return nc


def _get_nc():
    if "nc" not in _NC_CACHE:
        _NC_CACHE["nc"] = _build_nc()
    return _NC_CACHE["nc"]


def _etile_pack(wT):
    """[768, n] (e on rows) -> [128, 6*n] bf16: e-tiles as column blocks so
    the whole operand loads as ONE [128, n] DMA."""
    n = wT.shape[1]
    a = wT.reshape(NE, 128, n).transpose(1, 0, 2)
    return np.ascontiguousarray(a.reshape(128, NE * n)).astype(BF16)


def _x_block_pack(x_b):
    """[2048, 768] x -> [128, 4 * 6 * 512] bf16, seq-block-major: block n
    holds e-tiles of sequence rows n*512..(n+1)*512 as column slabs."""
    a = x_b.reshape(4, 512, NE, 128)          # n, s, e, p
    a = a.transpose(3, 0, 2, 1)               # p, n, e, s
    return np.ascontiguousarray(a.reshape(128, 4 * NE * 512)).astype(BF16)


def _pad_headsT(w_rows):
    """[384, 768] head rows -> zero-pad head dim 96->128 -> transpose -> [768, 512]."""
    p = np.zeros((HPC * HDP, EMB), np.float32)
    p.reshape(HPC, HDP, EMB)[:, :HD] = w_rows.reshape(HPC, HD, EMB)
    return np.ascontiguousarray(p.T)


def _pad_bias(b_rows):
    """[384] head bias -> [128, HPC] padded/transposed for per-partition add."""
    p = np.zeros((HPC, HDP), np.float32)
    p[:, :HD] = b_rows.reshape(HPC, HD)
    return np.ascontiguousarray(p.T)


def kernel(x, Wq, bq, Wk, bk, Wv, bv, Wo, bo):
    x = np.asarray(x, np.float32)
    Wq, bq = np.asarray(Wq, np.float32), np.asarray(bq, np.float32)
    Wk, bk = np.asarray(Wk, np.float32), np.asarray(bk, np.float32)
    Wv, bv = np.asarray(Wv, np.float32), np.asarray(bv, np.float32)
    Wo, bo = np.asarray(Wo, np.float32), np.asarray(bo, np.float32)

    nc = _get_nc()

    in_maps = []
    for c in range(NCORES):
        b, hg = divmod(c, 2)
        hs = slice(hg * HPC * HD, (hg + 1) * HPC * HD)
        woT = Wo[:, hs].T  # [384, 768]
        wo_pack = np.ascontiguousarray(
            woT.reshape(3, 128, EMB).transpose(1, 0, 2).reshape(128, 3 * EMB))
        wq_et = _etile_pack(_pad_headsT(Wq[hs])).reshape(128, NE, HPC, HDP)
        wk_et = _etile_pack(_pad_headsT(Wk[hs])).reshape(128, NE, HPC, HDP)
        in_maps.append({
            "xtp": _x_block_pack(x[b]),
            "wqp0": np.ascontiguousarray(
                wq_et[:, :, 0].reshape(128, NE * HDP)),
            "wqpr": np.ascontiguousarray(
                wq_et[:, :, 1:].reshape(128, NE * 3 * HDP)),
            "wkp0": np.ascontiguousarray(
                wk_et[:, :, 0].reshape(128, NE * HDP)),
            "wkpr": np.ascontiguousarray(
                wk_et[:, :, 1:].reshape(128, NE * 3 * HDP)),
            "wvp": _etile_pack(np.ascontiguousarray(Wv[hs].T)),
            "wop": wo_pack.astype(BF16),
            "bqp": _pad_bias(bq[hs]),
            "bkp": _pad_bias(bk[hs]),
            "identp": np.ascontiguousarray(np.eye(128, dtype=np.float32))
            .astype(BF16),
        })

    global LAST_RESULT
    trace = bool(int(os.environ.get("KERNEL_TRACE", "0")))
    tmpdir = os.environ.get("KERNEL_TRACE_DIR") or None
    res = run_bass_kernel_spmd(nc, in_maps, list(range(NCORES)), trace=trace,
                               tmpdir=tmpdir)
    LAST_RESULT = res

    out = np.empty((B, SEQ, EMB), np.float32)
    for b in range(B):
        out[b] = res.results[2 * b]["outp"] + res.results[2 * b + 1]["outp"]
    # bv enters each head's output additively (sum of softmax weights is 1),
    # and bo is a plain add: both fold into one constant vector.
    out += Wo @ bv + bo
    return out

